# revision 7
# baseline (speedup 1.0000x reference)
"""AnatomicalContrastiveLoss on 8 trn2 NeuronCores (Bass/Tile), v2.

Sharding: core c = (b = c//2, h = c%2) streams its 48MB half-volume shard
(probaT/yT/embT voxel-major) computing
  - ps_sums[f|count, c] = sum_v e65[v,f] * y[v,c]  (1024 PE matmuls, 16-row
    streams; e65 carries a ones column so counts ride in psum row 64)
  - w[v] = sum_c ln(p[v,c] + 1e-30)   (monotonic stand-in for prod_c p)

Top-k: per-partition-row top-8 of w gives 1024 candidates per half; ONE pair
AllGather shares candidate values AND voxel ids (8KB). Both cores then rank
the 2048-value union locally, scatter all 2048 candidates' (gather-id,
sub-row) into a [128,2] psum via one-hot matmuls, and dma_gather the top-100
rows from a FULL-volume [e|y] gather array (canonical f-order, identical on
both pair cores) - no second AllGather, both cores hold identical he/lab.

Tail: f-range is split across the pair (h=0 -> f 0:32, h=1 -> f 32:64) with
role-blended slices (no role-dependent addressing). Pair term per f is ONE
bf16 matmul (mask-1)^T @ E = E[lab_i]-Esum = -neg, then Ln(scale=-1,
bias=Ei) and a masked row-reduce. AllReduce(sums+counts) -> avg; final
scalar AllReduce with scale -1/B.
"""
import os
from contextlib import ExitStack
import numpy as np

B, C, F = 4, 16, 64
Fh = F // 2            # per-core f-range in the tail
V = 262144
Vh = V // 2            # per-core voxels
K = 100
TAU, THETA = 0.1, 0.9
GCOLS = 80             # gather row: 64 emb + 16 y
GE = 8 * GCOLS         # gather element: 8 voxel-rows = 640 f32 = 2560B
SUMN = 1040            # sums 1024 (f*16+c) | count 16
NCORES = 8
NT = 32                # pass2 tiles
TV = Vh // NT
NS = TV // 128
NTP = 8                # pass1 chunks
TVP = Vh // NTP
NSP = TVP // 128


def build_program(stage=None):
    from concourse import bass, bacc, tile, mybir, masks

    f32 = mybir.dt.float32
    bf16 = mybir.dt.bfloat16
    u32 = mybir.dt.uint32
    i32 = mybir.dt.int32
    i16 = mybir.dt.int16
    AF = mybir.ActivationFunctionType
    OP = mybir.AluOpType
    AX = mybir.AxisListType
    AP = bass.AP

    def rap(base, free_dims):
        return AP(base.tensor, base.offset, [list(base.ap[0])] + [list(d) for d in free_dims])

    STAGE = int(os.environ.get("KSTAGE", "9")) if stage is None else stage
    TSPLIT = int(os.environ.get("KSPLIT", "8"))
    NREP = int(os.environ.get("KREPEAT", "1"))
    NOCOLL = os.environ.get("KNOCOLL", "0") == "1"

    nc = bacc.Bacc(None, target_bir_lowering=False)
    ctx = ExitStack()

    pt = nc.dram_tensor("pt", [Vh, C], f32, kind="ExternalInput")
    yt = nc.dram_tensor("yt", [Vh, C], f32, kind="ExternalInput")
    et = nc.dram_tensor("et", [Vh, F + 1], f32, kind="ExternalInput")
    garr = nc.dram_tensor("garr", [V, GCOLS], f32, kind="ExternalInput")
    role = nc.dram_tensor("role", [1, 8], f32, kind="ExternalInput")
    out = nc.dram_tensor("out", [1, 1], f32, kind="ExternalOutput")

    idxdr = nc.dram_tensor("idxdr", [128], i16)
    sumin = nc.dram_tensor("sumin", [1, SUMN], f32)
    sumout = nc.dram_tensor("sumout", [1, SUMN], f32)
    valin = nc.dram_tensor("valin", [1, 2048], f32)
    valout = nc.dram_tensor("valout", [2, 2048], f32)
    labflat = nc.dram_tensor("labflat", [K], f32)
    hehalf = nc.dram_tensor("hehalf", [Fh * K], f32)
    avghalf = nc.dram_tensor("avghalf", [C * Fh], f32)
    lin = nc.dram_tensor("lin", [1, 8], f32)
    lout = nc.dram_tensor("lout", [1, 8], f32)

    ALL = [list(range(NCORES))]
    PAIRS = [[2 * i, 2 * i + 1] for i in range(B)]

    with tile.TileContext(nc) as tc:
        with (
            tc.tile_pool(name="stream", bufs=3) as stream,
            tc.tile_pool(name="pass1", bufs=8) as pass1,
            tc.tile_pool(name="persist", bufs=1) as persist,
            tc.tile_pool(name="small", bufs=2) as small,
            tc.tile_pool(name="psum", bufs=1, space="PSUM") as psum,
            tc.tile_pool(name="psx", bufs=6, space="PSUM") as psx,
        ):
          for _rep in range(NREP):
            ident = persist.tile([128, 128], f32)
            masks.make_identity(nc, ident[:])
            ones128 = persist.tile([128, 1], f32)
            nc.vector.memset(ones128[:], 1.0)
            iota128 = persist.tile([128, 128], f32)
            iot1 = small.tile([128, 128], i32, tag="iot1")
            nc.gpsimd.iota(iot1[:], pattern=[[1, 128]], base=0, channel_multiplier=0)
            nc.vector.tensor_copy(iota128[:], iot1[:])
            iota16 = persist.tile([128, 16], f32)
            ioti = small.tile([128, 16], i32, tag="ioti")
            nc.gpsimd.iota(ioti[:], pattern=[[1, 16]], base=0, channel_multiplier=0)
            nc.vector.tensor_copy(iota16[:], ioti[:])
            iotc = small.tile([C, K], i32, tag="iotc")
            nc.gpsimd.iota(iotc[:], pattern=[[0, K]], base=0, channel_multiplier=1)
            iotcf = persist.tile([C, K], f32)
            nc.vector.tensor_copy(iotcf[:], iotc[:])
            # role scalars: [h, 1-h, own_off, partner_off] broadcast to partitions
            rtile = persist.tile([128, 8], f32)
            nc.sync.dma_start(rtile[:], AP(role, 0, [[0, 128], [1, 8]]))
            rH = rtile[:, 0:1]
            rHc = rtile[:, 1:2]
            roffo_u = persist.tile([128, 1], u32)
            nc.vector.tensor_copy(roffo_u[:], rtile[:, 2:3])
            roffp_f = rtile[:, 3:4]

            # ---------------- streaming ----------------
            lnbias = persist.tile([128, 1], f32)
            nc.vector.memset(lnbias[:], 1e-30)
            w_sb = persist.tile([128, NTP, NSP], f32)
            ptiles = []
            for t in range(NTP):
                ptile = pass1.tile([128, NSP, C], f32, tag="p")
                qp = nc.sync if t % 2 == 0 else nc.scalar
                qp.dma_start(ptile[:], AP(pt, t * TVP * C, [[NSP * C, 128], [C, NSP], [1, C]]))
                ptiles.append(ptile)

            def w_chunk(t):
                lnp = ptiles[t][:].rearrange('p a b -> p (a b)')
                nc.scalar.activation(lnp, lnp, AF.Ln, bias=lnbias[:])
                nc.vector.tensor_reduce(w_sb[:, t, :], rap(lnp, [[C, NSP], [1, C]]), AX.X, OP.add)

            ps_sums = psum.tile([C, F + 1], f32, tag="sums")
            def stream_tile(t):
                ytile = stream.tile([128, NS, C], f32, tag="y")
                etile = stream.tile([128, NS, F + 1], f32, tag="e")
                qy = nc.scalar if t % 2 == 0 else nc.sync
                qy.dma_start(ytile[:], AP(yt, t * TV * C, [[NS * C, 128], [C, NS], [1, C]]))
                q = nc.sync if t % 2 == 0 else nc.scalar
                q.dma_start(etile[:], AP(et, t * TV * (F + 1), [[NS * (F + 1), 128], [F + 1, NS], [1, F + 1]]))
                for s in range(NS):
                    nc.tensor.matmul(
                        ps_sums[:], ytile[:, s, :], etile[:, s, :],
                        start=(t == 0 and s == 0), stop=(t == NT - 1 and s == NS - 1),
                    )

            for t in range(TSPLIT):
                stream_tile(t)
                if t < NTP:
                    w_chunk(t)
            for t in range(TSPLIT, NTP):
                w_chunk(t)

            # ---------------- chain A: top-k candidates + pair AG + ranks ----
            if STAGE >= 5:
                mxpack = persist.tile([128, 16], f32)
                mx8 = mxpack[:, 0:8]
                nc.vector.max(mx8, w_sb[:].rearrange('p a b -> p (a b)'))
                mi8 = persist.tile([128, 8], u32)
                nc.vector.max_index(mi8[:], mx8, w_sb[:].rearrange('p a b -> p (a b)'))
                # (mxpack cols 8:16 hold voxel ids as f32 for the pair AG)
                # v_local = ((mi>>7)<<14) + p*128 + (mi&127)
                vglob = persist.tile([128, 8], u32)
                nc.vector.tensor_scalar(vglob[:], mi8[:], 7, None, OP.logical_shift_right)
                nc.vector.tensor_scalar(vglob[:], vglob[:], 14, None, OP.logical_shift_left)
                tmp8 = small.tile([128, 8], u32, tag="tmp8")
                nc.vector.tensor_scalar(tmp8[:], mi8[:], 127, None, OP.bitwise_and)
                nc.vector.tensor_tensor(vglob[:], vglob[:], tmp8[:], OP.add)
                prow = small.tile([128, 1], i32, tag="prow")
                nc.gpsimd.iota(prow[:], pattern=[[0, 1]], base=0, channel_multiplier=128)
                prowu = small.tile([128, 1], u32, tag="prowu")
                nc.vector.tensor_copy(prowu[:], prow[:])
                nc.vector.tensor_tensor(vglob[:], vglob[:], rap(prowu[:], [[0, 8]]), OP.add)
                # own global ids + sc2o = (vg//8+1, vg%8+1)
                vgo = small.tile([128, 8], u32, tag="vgo")
                nc.vector.tensor_tensor(vgo[:], vglob[:], rap(roffo_u[:], [[0, 8]]), OP.add)
                i4o = small.tile([128, 8], u32, tag="i4o")
                nc.vector.tensor_scalar(i4o[:], vgo[:], 3, None, OP.logical_shift_right)
                sbo = small.tile([128, 8], u32, tag="sbo")
                nc.vector.tensor_scalar(sbo[:], vgo[:], 7, None, OP.bitwise_and)
                sc2o = persist.tile([128, 8, 2], f32)
                nc.vector.tensor_copy(rap(sc2o[:], [[2, 8]]), i4o[:])
                nc.vector.tensor_copy(AP(sc2o.tensor, sc2o[:].offset + 1, [list(sc2o[:].ap[0]), [2, 8]]), sbo[:])
                nc.vector.tensor_scalar(sc2o[:], sc2o[:], 1.0, None, OP.add)

                # pair AllGather of [vals | voxel ids]
                nc.vector.tensor_copy(mxpack[:, 8:16], vglob[:])
                nc.gpsimd.dma_start(AP(valin, 0, [[8, 128], [1, 8]]), mxpack[:, 0:8])
                nc.gpsimd.dma_start(AP(valin, 1024, [[8, 128], [1, 8]]), mxpack[:, 8:16])
                NOAG = os.environ.get("KNOAG", "0") == "1" or NOCOLL
                vsrc, voff = (valin, 0) if NOAG else (valout, 2048)
                if not NOAG:
                    nc.gpsimd.collective_compute("AllGather", OP.bypass, ins=[valin[:]], outs=[valout[:]], replica_groups=PAIRS)
                unionA = persist.tile([128, 1024], f32)
                unionB = persist.tile([128, 1024], f32)
                nc.gpsimd.dma_start(unionA[:], AP(vsrc, 0, [[0, 128], [1, 1024]]))
                nc.gpsimd.dma_start(unionB[:], AP(vsrc, voff, [[0, 128], [1, 1024]]))
                rvb = small.tile([128, 2, 2, 8], f32, tag="rvb")
                if NOAG:
                    nc.gpsimd.dma_start(rvb[:, 0, :, :], AP(vsrc, 0, [[8, 128], [1024, 2], [1, 8]]))
                    nc.gpsimd.dma_start(rvb[:, 1, :, :], AP(vsrc, 0, [[8, 128], [1024, 2], [1, 8]]))
                else:
                    nc.gpsimd.dma_start(rvb[:], AP(vsrc, 0, [[8, 128], [voff, 2], [1024, 2], [1, 8]]))
                rv0 = rvb[:, 0, 0, :]
                rg0 = rvb[:, 0, 1, :]
                rv1 = rvb[:, 1, 0, :]
                rg1 = rvb[:, 1, 1, :]
                # partner candidates: h=0's partner is AG row 1, h=1's is row 0
                pmx8 = persist.tile([128, 8], f32)
                nc.vector.tensor_scalar(pmx8[:], rv0, rH, None, OP.mult)
                nc.vector.scalar_tensor_tensor(pmx8[:], rv1, rHc, pmx8[:], OP.mult, OP.add)
                pvgf = persist.tile([128, 8], f32)
                nc.vector.tensor_scalar(pvgf[:], rg0, rH, None, OP.mult)
                nc.vector.scalar_tensor_tensor(pvgf[:], rg1, rHc, pvgf[:], OP.mult, OP.add)
                # partner sc2
                nc.vector.tensor_tensor(pvgf[:], pvgf[:], rap(roffp_f, [[0, 8]]), OP.add)
                pvgu = small.tile([128, 8], u32, tag="pvgu")
                nc.vector.tensor_copy(pvgu[:], pvgf[:])
                i4p = small.tile([128, 8], u32, tag="i4p")
                nc.vector.tensor_scalar(i4p[:], pvgu[:], 3, None, OP.logical_shift_right)
                sbp = small.tile([128, 8], u32, tag="sbp")
                nc.vector.tensor_scalar(sbp[:], pvgu[:], 7, None, OP.bitwise_and)
                sc2p = persist.tile([128, 8, 2], f32)
                nc.vector.tensor_copy(rap(sc2p[:], [[2, 8]]), i4p[:])
                nc.vector.tensor_copy(AP(sc2p.tensor, sc2p[:].offset + 1, [list(sc2p[:].ap[0]), [2, 8]]), sbp[:])
                nc.vector.tensor_scalar(sc2p[:], sc2p[:], 1.0, None, OP.add)
                # ranks in the 2048-union (w_sb storage is dead -> scratch)
                geb = w_sb[:].rearrange('p a b -> p (a b)')
                rkA = small.tile([128, 8], f32, tag="rkA")
                rkB = small.tile([128, 8], f32, tag="rkB")
                for c8 in range(8):
                    nc.vector.tensor_scalar(geb, unionA[:], mxpack[:, c8:c8 + 1], None, OP.is_ge,
                                            OP.add, accum_out=rkA[:, c8:c8 + 1])
                for c8 in range(8):
                    nc.vector.tensor_scalar(geb, unionB[:], mxpack[:, c8:c8 + 1], None, OP.is_ge,
                                            OP.add, accum_out=rkB[:, c8:c8 + 1])
                slotfo = persist.tile([128, 8], f32)
                nc.vector.tensor_tensor(slotfo[:], rkA[:], rkB[:], OP.add)
                nc.vector.tensor_scalar(slotfo[:], slotfo[:], -1.0, None, OP.add)
                for c8 in range(8):
                    nc.vector.tensor_scalar(geb, unionA[:], pmx8[:, c8:c8 + 1], None, OP.is_ge,
                                            OP.add, accum_out=rkA[:, c8:c8 + 1])
                for c8 in range(8):
                    nc.vector.tensor_scalar(geb, unionB[:], pmx8[:, c8:c8 + 1], None, OP.is_ge,
                                            OP.add, accum_out=rkB[:, c8:c8 + 1])
                slotfp = persist.tile([128, 8], f32)
                nc.vector.tensor_tensor(slotfp[:], rkA[:], rkB[:], OP.add)
                nc.vector.tensor_scalar(slotfp[:], slotfp[:], -1.0, None, OP.add)

            # ---------------- streaming, rest ----------------
            KS2 = int(os.environ.get("KSPLIT2", "24"))
            KS3 = int(os.environ.get("KSPLIT3", "28"))
            for t in range(TSPLIT, KS2):
                stream_tile(t)

            # ---- chain B1 (mid-stream): scatter matmuls + gather ids ----
            # PE reaches these matmuls around tile KS2, by which time the
            # slot data (DVE ranks, gated on the pair AG) is long ready, so
            # no engine stalls and everything downstream is mid-stream too.
            if STAGE >= 5:
                ps_sc = psum.tile([128, 2], f32, tag="mm")
                oh16 = persist.tile([128, 16, 128], f32)
                for c8 in range(8):
                    nc.vector.tensor_scalar(oh16[:, c8, :], iota128[:], slotfo[:, c8:c8 + 1], None, OP.is_equal)
                for c8 in range(8):
                    nc.vector.tensor_scalar(oh16[:, 8 + c8, :], iota128[:], slotfp[:, c8:c8 + 1], None, OP.is_equal)
                for c8 in range(8):
                    nc.tensor.matmul(ps_sc[:], oh16[:, c8, :], sc2o[:, c8, :], start=(c8 == 0), stop=False)
                for c8 in range(8):
                    nc.tensor.matmul(ps_sc[:], oh16[:, 8 + c8, :], sc2p[:, c8, :], start=False, stop=(c8 == 7))
                gslot = persist.tile([128, 2], f32)
                nc.vector.tensor_copy(gslot[:], ps_sc[:])
                subcol = persist.tile([128, 1], f32)
                nc.vector.tensor_scalar(subcol[:], gslot[:, 1:2], -1.0, None, OP.add)
                gidxf = small.tile([128, 1], f32, tag="gidxf")
                nc.vector.tensor_scalar(gidxf[:], gslot[:, 0:1], -1.0, 0.0, OP.add, OP.max)
                gidxi = small.tile([128, 1], i16, tag="gidxi")
                nc.vector.tensor_copy(gidxi[:], gidxf[:])

            for t in range(KS2, KS3):
                stream_tile(t)

            # ---- chain B2 (late-mid-stream): idx round-trip + dma_gather ----
            if STAGE >= 5:
                nc.sync.dma_start(AP(idxdr, 0, [[1, 128]]), gidxi[:])
                gidx = small.tile([128, 8], i16, tag="gidx")
                for blk in range(8):
                    qg = nc.sync if blk % 2 == 0 else nc.scalar
                    qg.dma_start(gidx[blk * 16:(blk + 1) * 16, :], AP(idxdr, 0, [[1, 16], [16, 8]]))
                gg = persist.tile([128, 1, GE], f32)
                gin = AP(garr, 0, [[GE, V // 8], [1, GE]])
                nc.gpsimd.dma_gather(gg[:], gin, gidx[:], num_idxs=128, num_idxs_reg=128, elem_size=GE)
                cand = persist.tile([128, GCOLS], f32)
                ohall = small.tile([128, 8], f32, tag="ohl")
                for r in range(8):
                    nc.vector.tensor_scalar(ohall[:, r:r + 1], subcol[:], float(r), None, OP.is_equal)
                nc.vector.tensor_scalar(cand[:], gg[:, 0, 0:GCOLS], ohall[:, 0:1], None, OP.mult)
                for r in range(1, 8):
                    nc.vector.scalar_tensor_tensor(cand[:], gg[:, 0, r * GCOLS:(r + 1) * GCOLS],
                                                   ohall[:, r:r + 1], cand[:], OP.mult, OP.add)
                labm = small.tile([128, 16], f32, tag="labm")
                nc.vector.tensor_tensor(labm[:], cand[:, F:GCOLS], iota16[:], OP.mult)
                labmine = persist.tile([128, 1], f32)
                nc.vector.tensor_reduce(labmine[:], labm[:], AX.X, OP.add)
                labP = labmine[0:K, :]
                heP = cand[0:K, 0:F]
                hePH = persist.tile([K, Fh], f32)
                nc.vector.tensor_scalar(hePH[:], heP[:, 0:Fh], rHc[0:K, :], None, OP.mult)
                nc.vector.scalar_tensor_tensor(hePH[:], heP[:, Fh:F], rH[0:K, :], hePH[:], OP.mult, OP.add)

            for t in range(KS3, NT):
                stream_tile(t)

            # ---------------- sums epilogue + AllReduce ----------------
            sums_sb = persist.tile([C, F + 1], f32)
            nc.scalar.activation(sums_sb[:], ps_sums[:], AF.Copy)
            nc.sync.dma_start(AP(sumin, 0, [[F, C], [1, F]]), sums_sb[:, 0:F])
            nc.sync.dma_start(AP(sumin, 1024, [[1, C]]), sums_sb[:, F:F + 1])
            if STAGE >= 1:
                if NOCOLL:
                    nc.gpsimd.dma_start(sumout[:], sumin[:])
                else:
                    nc.gpsimd.collective_compute("AllReduce", OP.add, ins=[sumin[:]], outs=[sumout[:]], replica_groups=ALL)

            # ---- chain B3 (post-stream, overlaps the AllReduce) ----
            if STAGE >= 5:
                ps_hes = psum.tile([Fh, K], f32, tag="mm")
                nc.tensor.transpose(ps_hes[:], hePH[:], ident[0:K, 0:K])
                hh = persist.tile([Fh, K], f32)
                nc.vector.tensor_copy(hh[:], ps_hes[:])
                nc.sync.dma_start(AP(hehalf, 0, [[K, Fh], [1, K]]), hh[:])
                he_rep = persist.tile([C, Fh * K], f32)
                nc.sync.dma_start(he_rep[:], AP(hehalf, 0, [[0, C], [1, Fh * K]]))
                nc.sync.dma_start(AP(labflat, 0, [[1, K]]), labP)
                labrep16 = small.tile([C, K], f32, tag="lr16")
                nc.sync.dma_start(labrep16[:], AP(labflat, 0, [[0, C], [1, K]]))
                Mp = persist.tile([C, K], f32)
                nc.vector.tensor_tensor(Mp[:], labrep16[:], iotcf[:], OP.is_equal)
                Mm1 = persist.tile([C, K], bf16)
                nc.vector.tensor_scalar(Mm1[:], Mp[:], -1.0, None, OP.add)
                M_kc = persist.tile([K, C], f32)
                nc.vector.tensor_scalar(M_kc[:], iota16[0:K, :], labP, None, OP.is_equal)
                labrepK = small.tile([K, K], f32, tag="lrK")
                nc.sync.dma_start(labrepK[:], AP(labflat, 0, [[0, K], [1, K]]))
                mask2 = persist.tile([K, K], f32)
                nc.vector.tensor_scalar(mask2[:], labrepK[:], labP, None, OP.is_equal)

                nk = small.tile([C, 1], f32, tag="nk")
                nc.vector.tensor_reduce(nk[:], Mp[:], AX.X, OP.add)
                nk2 = small.tile([C, 1], f32, tag="nk2")
                nc.vector.tensor_tensor(nk2[:], nk[:], nk[:], OP.mult)
                den = small.tile([C, 1], f32, tag="den")
                nc.vector.tensor_scalar(den[:], nk2[:], float(F), 1.0, OP.mult, OP.max)
                wc0 = small.tile([C, 1], f32, tag="wc0")
                nc.vector.reciprocal(wc0[:], den[:])
                gnk = small.tile([C, 1], f32, tag="gnk")
                nc.vector.tensor_scalar(gnk[:], nk[:], 0.0, None, OP.is_gt)
                rhs2 = persist.tile([C, 2], f32)
                nc.vector.tensor_tensor(rhs2[:, 0:1], wc0[:], gnk[:], OP.mult)
                nc.vector.tensor_tensor(rhs2[:, 1:2], rhs2[:, 0:1], nk[:], OP.mult)

            # ---------------- post-AR tail ----------------
            if STAGE >= 6:
                tot = persist.tile([C, F], f32)
                nc.sync.dma_start(tot[:], AP(sumout, 0, [[F, C], [1, F]]))
                totc = small.tile([C, 1], f32, tag="totc")
                nc.sync.dma_start(totc[:], AP(sumout, 1024, [[1, C]]))
                cmax = small.tile([C, 1], f32, tag="cmax")
                nc.vector.tensor_scalar(cmax[:], totc[:], 1.0, None, OP.max)
                crec = small.tile([C, 1], f32, tag="crec")
                nc.vector.reciprocal(crec[:], cmax[:])
                cgt = small.tile([C, 1], f32, tag="cgt")
                nc.vector.tensor_scalar(cgt[:], totc[:], 0.0, None, OP.is_gt)
                csc = small.tile([C, 1], f32, tag="csc")
                nc.vector.tensor_scalar(csc[:], crec[:], cgt[:], THETA, OP.mult, OP.mult)
                avg = persist.tile([C, F], f32)
                nc.vector.tensor_scalar(avg[:], tot[:], csc[:], None, OP.mult)
                avgH = persist.tile([C, Fh], f32)
                nc.vector.tensor_scalar(avgH[:], avg[:, 0:Fh], rHc[0:C, :], None, OP.mult)
                nc.vector.scalar_tensor_tensor(avgH[:], avg[:, Fh:F], rH[0:C, :], avgH[:], OP.mult, OP.add)
                nc.sync.dma_start(AP(avghalf, 0, [[Fh, C], [1, Fh]]), avgH[:])
                avgrep = persist.tile([K, C * Fh], f32)
                nc.sync.dma_start(avgrep[:], AP(avghalf, 0, [[0, K], [1, C * Fh]]))
                ps_avt = psum.tile([Fh, C], f32, tag="mm")
                nc.tensor.transpose(ps_avt[:], avgH[:], ident[0:C, 0:C])
                avgHT = persist.tile([Fh, C], f32)
                nc.vector.tensor_copy(avgHT[:], ps_avt[:])

                # Ep[c, f*K+j] = exp(avg[c,f]*he[f,j]/tau), bf16
                Ep = persist.tile([C, Fh * K], bf16)
                ECH = Fh * K // 4
                for ch in range(4):
                    sl = slice(ch * ECH, (ch + 1) * ECH)
                    nc.vector.tensor_tensor(
                        Ep[:, sl], he_rep[:, sl],
                        AP(avgH.tensor, avgH[:].offset + ch * (Fh // 4), [list(avgH[:].ap[0]), [1, Fh // 4], [0, K]]),
                        OP.mult)
                    nc.scalar.activation(Ep[:, sl], Ep[:, sl], AF.Exp, scale=1.0 / TAU)

                # Ei[i, f] = exp(he_i[f]*avg[lab_i,f]/tau)
            if STAGE >= 7:
                ekparg = persist.tile([K, C * Fh], f32)
                nc.vector.tensor_tensor(ekparg[:], rap(hePH[:], [[0, C], [1, Fh]]), avgrep[:], OP.mult)
                nc.scalar.activation(ekparg[:], ekparg[:], AF.Exp, scale=1.0 / TAU)
                nc.vector.tensor_tensor(ekparg[:], ekparg[:], rap(M_kc[:], [[1, C], [0, Fh]]), OP.mult)
                Ei = persist.tile([K, Fh], f32)
                nc.vector.tensor_reduce(Ei[:], rap(ekparg[:], [[1, Fh], [Fh, C]]), AX.X, OP.add)

                # pair loop over own f-half
                USE_TTR = os.environ.get("KTTR", "0") == "1"
                S2cols = persist.tile([K, Fh], f32)
                scratch = persist.tile([K, K], f32)
                if USE_TTR:
                    for fl in range(Fh):
                        ps_x = psx.tile([K, K], f32, tag="x")
                        nc.tensor.matmul(ps_x[:], Mm1[:], Ep[:, fl * K:(fl + 1) * K], start=True, stop=True)
                        termf = small.tile([K, K], f32, tag="termf")
                        nc.scalar.activation(termf[:], ps_x[:], AF.Ln, scale=-1.0, bias=Ei[:, fl:fl + 1])
                        nc.vector.tensor_tensor_reduce(
                            scratch[:], termf[:], mask2[:], 1.0, 0.0, OP.mult, OP.add,
                            accum_out=S2cols[:, fl:fl + 1])
                    S2 = small.tile([K, 1], f32, tag="S2")
                    nc.vector.tensor_reduce(S2[:], S2cols[:], AX.X, OP.add)
                else:
                    pairacc = persist.tile([K, K], f32)
                    for fl in range(Fh):
                        ps_x = psx.tile([K, K], f32, tag="x")
                        nc.tensor.matmul(ps_x[:], Mm1[:], Ep[:, fl * K:(fl + 1) * K], start=True, stop=True)
                        termf = small.tile([K, K], f32, tag="termf")
                        nc.scalar.activation(termf[:], ps_x[:], AF.Ln, scale=-1.0, bias=Ei[:, fl:fl + 1])
                        if fl == 0:
                            nc.vector.tensor_copy(pairacc[:], termf[:])
                        else:
                            nc.vector.tensor_tensor(pairacc[:], pairacc[:], termf[:], OP.add)
                    sm = small.tile([K, K], f32, tag="sm")
                    nc.vector.tensor_tensor(sm[:], pairacc[:], mask2[:], OP.mult)
                    S2 = small.tile([K, 1], f32, tag="S2")
                    nc.vector.tensor_reduce(S2[:], sm[:], AX.X, OP.add)

            if STAGE >= 8:
                ps_u = psum.tile([K, 2], f32, tag="mm")
                nc.tensor.matmul(ps_u[:], Mp[:], rhs2[:], start=True, stop=True)
                U = small.tile([K, 2], f32, tag="U")
                nc.vector.tensor_copy(U[:], ps_u[:])
                ps_g = psum.tile([K, C], f32, tag="mm")
                nc.tensor.matmul(ps_g[:], hh[:], avgHT[:], start=True, stop=True)
                Gm = small.tile([K, C], f32, tag="Gm")
                nc.vector.tensor_tensor(Gm[:], ps_g[:], M_kc[:], OP.mult)
                li = small.tile([K, 1], f32, tag="li")
                nc.vector.tensor_reduce(li[:], Gm[:], AX.X, OP.add)

                t1 = small.tile([K, 1], f32, tag="t1")
                nc.vector.tensor_tensor(t1[:], S2[:], U[:, 0:1], OP.mult)
                t2 = small.tile([K, 1], f32, tag="t2")
                nc.vector.tensor_tensor(t2[:], li[:], U[:, 1:2], OP.mult)
                cvec = small.tile([K, 1], f32, tag="cvec")
                nc.vector.scalar_tensor_tensor(cvec[:], t2[:], -1.0 / TAU, t1[:], OP.mult, OP.add)

                ps_t = psum.tile([1, 1], f32, tag="mm")
                nc.tensor.matmul(ps_t[:], cvec[:], ones128[0:K, :], start=True, stop=True)
                lossp = small.tile([1, 8], f32, tag="lossp")
                nc.vector.memset(lossp[:], 0.0)
                nc.scalar.activation(lossp[:, 0:1], ps_t[:], AF.Copy, scale=-1.0 / B)
                nc.sync.dma_start(lin[:], lossp[:])
                if NOCOLL:
                    nc.gpsimd.dma_start(lout[:], lin[:])
                else:
                    nc.gpsimd.collective_compute("AllReduce", OP.add, ins=[lin[:]], outs=[lout[:]], replica_groups=ALL)
                res = small.tile([1, 1], f32, tag="res")
                nc.sync.dma_start(res[:], lout[0:1, 0:1])
                nc.sync.dma_start(out[:], res[:])

            if STAGE < 8:
                dbg = small.tile([1, 1], f32, tag="dbg")
                if STAGE == 0:
                    nc.sync.dma_start(dbg[:], AP(sumin, 0, [[1, 1]]))
                elif STAGE == 1:
                    nc.sync.dma_start(dbg[:], AP(sumout, 0, [[1, 1]]))
                elif STAGE == 5:
                    nc.sync.dma_start(dbg[:], AP(labflat, 0, [[1, 1]]))
                elif STAGE == 6:
                    nc.vector.tensor_copy(dbg[:], Ep[0:1, 0:1])
                elif STAGE == 7:
                    nc.vector.tensor_copy(dbg[:], S2[0:1, 0:1])
                nc.sync.dma_start(out[:], dbg[:])

    nc.compile()
    ctx.close()
    return nc


def make_in_maps(proba, y, embeddings):
    in_maps = []
    for core in range(NCORES):
        b, h = core // 2, core % 2
        sl = slice(h * Vh, (h + 1) * Vh)
        pT = np.ascontiguousarray(proba[b, :, sl].T)
        yT = np.ascontiguousarray(y[b, :, sl].T)
        eT = np.ascontiguousarray(
            np.concatenate([embeddings[b, :, sl].T,
                            np.ones((Vh, 1), np.float32)], axis=1))
        ga = np.ascontiguousarray(
            np.concatenate([embeddings[b].T, y[b].T], axis=1).astype(np.float32))
        rl = np.array([[h, 1 - h, h * Vh, (1 - h) * Vh, 0, 0, 0, 0]], dtype=np.float32)
        in_maps.append({"pt": pT, "yt": yT, "et": eT, "garr": ga, "role": rl})
    return in_maps


_NC = None


def kernel(proba, y, embeddings):
    global _NC
    from concourse.bass_utils import run_bass_kernel_spmd

    if _NC is None:
        _NC = build_program()
    in_maps = make_in_maps(proba, y, embeddings)
    res = run_bass_kernel_spmd(_NC, in_maps, core_ids=list(range(NCORES)))
    return np.float32(res.results[0]["out"].reshape(())).reshape(())



# revision 45
# speedup vs baseline: 2.3070x; 2.3070x over previous
"""AnatomicalContrastiveLoss on 8 trn2 NeuronCores (Bass/Tile), v3.

Sharding: core c = (b = c//2, h = c%2) streams its half-volume shard
(probaT f32 / yT bf16 / embT bf16, voxel-major; e/y host-cast to bf16 to
halve HBM traffic) computing
  - ps_sums[c, f|count] = sum_v y[v,c] * e65[v,f]  (1024 bf16 PE matmuls;
    e65 carries a ones column so counts ride in psum col 64)
  - w[v] = sum_c ln(p[v,c] + 1e-30)  (monotonic stand-in for prod_c p)

Top-k: per-partition-row top-8 of w gives 1024 candidates per half; ONE pair
AllGather shares candidate values AND voxel ids (8KB). Both cores rank the
2048-value union locally, scatter all 2048 candidates' (gather-id, sub-row)
into a [128,2] psum via one-hot matmuls, and dma_gather the top-100 rows
from a FULL-volume [e|y] gather array (identical on both pair cores).

Tail: f-range split across the pair (h=0 -> f 0:32, h=1 -> f 32:64) with
role-blended slices. Class sums/counts exchanged via AllGather + local
reduce (fewer ring steps than AllReduce). Broadcasts built on-chip via
ones-column matmuls (no DRAM round trips except he_rep, hidden mid-stream).
Pair term in NQ chunks: matmul (mask-1)^T @ Ep = -neg, DVE subtract with Ei
broadcast, one Ln per chunk, masked reduce. Final scalar via AllGather +
local sum.
"""
import os
from contextlib import ExitStack

import numpy as np

B, C, F = 4, 16, 64
Fh = F // 2            # per-core f-range in the tail
V = 262144
Vh = V // 2            # per-core voxels
K = 100
TAU, THETA = 0.1, 0.9
GCOLS = 80             # gather row: 64 emb + 16 y
GE = 8 * GCOLS         # gather element: 8 voxel-rows = 640 f32 = 2560B
SUMN = 1040            # sums 1024 (c*64+f) | count 16
NCORES = 8
NT = 16                # stream tiles
TV = Vh // NT
NS = TV // 128
NTP = 8                # pass1 (proba) chunks
TVP = Vh // NTP
NSP = TVP // 128
NQ = 4                 # tail chunks (8 f-slices each)
FQ = Fh // NQ


def build_program(stage=None):
    from concourse import bass, bacc, tile, mybir, masks

    f32 = mybir.dt.float32
    bf16 = mybir.dt.bfloat16
    u32 = mybir.dt.uint32
    i32 = mybir.dt.int32
    i16 = mybir.dt.int16
    AF = mybir.ActivationFunctionType
    OP = mybir.AluOpType
    AX = mybir.AxisListType
    AP = bass.AP

    def rap(base, free_dims):
        return AP(base.tensor, base.offset, [list(base.ap[0])] + [list(d) for d in free_dims])

    TSPLIT = int(os.environ.get("KSPLIT", "8"))
    KS2 = int(os.environ.get("KSPLIT2", "10"))
    KS3 = int(os.environ.get("KSPLIT3", "11"))
    KS4 = int(os.environ.get("KSPLIT4", "13"))
    NREP = int(os.environ.get("KREPEAT", "1"))
    NOCOLL = os.environ.get("KNOCOLL", "0") == "1"
    KHW = int(os.environ.get("KHW", "6"))

    nc = bacc.Bacc(None, target_bir_lowering=False)
    ctx = ExitStack()

    pt = nc.dram_tensor("pt", [Vh, C], bf16, kind="ExternalInput")
    yt = nc.dram_tensor("yt", [Vh, C], bf16, kind="ExternalInput")
    et = nc.dram_tensor("et", [Vh, F + 1], bf16, kind="ExternalInput")
    garr = nc.dram_tensor("garr", [V, GCOLS], f32, kind="ExternalInput")
    role = nc.dram_tensor("role", [1, 8], f32, kind="ExternalInput")
    out = nc.dram_tensor("out", [1, 1], f32, kind="ExternalOutput")

    idxdr = nc.dram_tensor("idxdr", [128], i16)
    hehalf = nc.dram_tensor("hehalf", [Fh * K], f32)
    sumin = nc.dram_tensor("sumin", [1, SUMN], f32)
    sumout = nc.dram_tensor("sumout", [8, SUMN], f32)
    valin = nc.dram_tensor("valin", [1, 2048], f32)
    valout = nc.dram_tensor("valout", [2, 2048], f32)
    lin = nc.dram_tensor("lin", [1, 8], f32)
    lout = nc.dram_tensor("lout", [8, 8], f32)

    ALL = [list(range(NCORES))]
    PAIRS = [[2 * i, 2 * i + 1] for i in range(B)]

    with tile.TileContext(nc) as tc:
        with (
            tc.tile_pool(name="stream", bufs=4) as stream,
            tc.tile_pool(name="pass1", bufs=8) as pass1,
            tc.tile_pool(name="persist", bufs=1) as persist,
            tc.tile_pool(name="small", bufs=2) as small,
            tc.tile_pool(name="psum", bufs=1, space="PSUM") as psum,
            tc.tile_pool(name="psx", bufs=2, space="PSUM") as psx,
        ):
          for _rep in range(NREP):
            ident = persist.tile([128, 128], f32)
            masks.make_identity(nc, ident[:])
            ones128 = persist.tile([128, 1], f32)
            nc.vector.memset(ones128[:], 1.0)
            onesr = persist.tile([1, 128], bf16)
            nc.vector.memset(onesr[:], 1.0)
            iota128 = persist.tile([128, 128], f32)
            iot1 = small.tile([128, 128], i32, tag="iot1")
            nc.gpsimd.iota(iot1[:], pattern=[[1, 128]], base=0, channel_multiplier=0)
            nc.vector.tensor_copy(iota128[:], iot1[:])
            iota16 = persist.tile([128, 16], f32)
            ioti = small.tile([128, 16], i32, tag="ioti")
            nc.gpsimd.iota(ioti[:], pattern=[[1, 16]], base=0, channel_multiplier=0)
            nc.vector.tensor_copy(iota16[:], ioti[:])
            iotc = small.tile([C, K], i32, tag="iotc")
            nc.gpsimd.iota(iotc[:], pattern=[[0, K]], base=0, channel_multiplier=1)
            iotcf = persist.tile([C, K], f32)
            nc.vector.tensor_copy(iotcf[:], iotc[:])
            # role scalars: [h, 1-h, own_off, partner_off] broadcast to partitions
            rtile = persist.tile([128, 8], f32)
            nc.sync.dma_start(rtile[:], AP(role, 0, [[0, 128], [1, 8]]))
            rH = rtile[:, 0:1]
            rHc = rtile[:, 1:2]
            roffo_u = persist.tile([128, 1], u32)
            nc.vector.tensor_copy(roffo_u[:], rtile[:, 2:3])
            roffp_f = rtile[:, 3:4]
            # preload the Exp activation table while ACT is idle
            actwarm = persist.tile([1, 1], f32)
            nc.scalar.activation(actwarm[:], ones128[0:1, :], AF.Exp)

            # ---------------- streaming ----------------
            lnbias = persist.tile([128, 1], f32)
            nc.vector.memset(lnbias[:], 1e-30)
            w_sb = persist.tile([128, NTP, NSP], f32)
            ptiles = []
            for t in range(NTP):
                ptile = pass1.tile([128, NSP, C], bf16, tag="p")
                qp = nc.sync if t % 2 == 0 else nc.scalar
                qp.dma_start(ptile[:], AP(pt, t * TVP * C, [[NSP * C, 128], [C, NSP], [1, C]]))
                ptiles.append(ptile)

            def w_chunk(t):
                # w = sum_c ln(p + 1e-30): monotonic stand-in for prod_c p
                # (p is bf16; Ln output must stay f32 for ranking precision)
                pf = ptiles[t][:].rearrange('p a b -> p (a b)')
                lnp = pass1.tile([128, NSP * C], f32, tag="lnp", bufs=2)
                nc.scalar.activation(lnp[:], pf, AF.Ln, bias=lnbias[:])
                nc.vector.tensor_reduce(w_sb[:, t, :], rap(lnp[:], [[C, NSP], [1, C]]), AX.X, OP.add)

            ps_sums = psum.tile([C, F + 1], f32, tag="sums")
            def stream_tile(t):
                ytile = stream.tile([128, NS, C], bf16, tag="y")
                etile = stream.tile([128, NS, F + 1], bf16, tag="e")
                qy = nc.scalar if t % 2 == 0 else nc.sync
                qy.dma_start(ytile[:], AP(yt, t * TV * C, [[NS * C, 128], [C, NS], [1, C]]))
                q = nc.sync if t % 2 == 0 else nc.scalar
                q.dma_start(etile[:], AP(et, t * TV * (F + 1), [[NS * (F + 1), 128], [F + 1, NS], [1, F + 1]]))
                for s in range(NS):
                    nc.tensor.matmul(
                        ps_sums[:], ytile[:, s, :], etile[:, s, :],
                        start=(t == 0 and s == 0), stop=(t == NT - 1 and s == NS - 1),
                    )

            for t in range(TSPLIT):
                stream_tile(t)
                if t < NTP:
                    w_chunk(t)
            for t in range(TSPLIT, NTP):
                w_chunk(t)

            # ---------------- chain A: top-k candidates + pair AG + ranks ----
            if KHW >= 2:
                mxpack = persist.tile([128, 16], f32)
                mx8 = mxpack[:, 0:8]
                nc.vector.max(mx8, w_sb[:].rearrange('p a b -> p (a b)'))
                mi8 = persist.tile([128, 8], u32)
                nc.vector.max_index(mi8[:], mx8, w_sb[:].rearrange('p a b -> p (a b)'))
                # v_local = ((mi>>7)<<14) + p*128 + (mi&127)
                vglob = persist.tile([128, 8], u32)
                nc.vector.tensor_scalar(vglob[:], mi8[:], 7, None, OP.logical_shift_right)
                nc.vector.tensor_scalar(vglob[:], vglob[:], 14, None, OP.logical_shift_left)
                tmp8 = small.tile([128, 8], u32, tag="tmp8")
                nc.vector.tensor_scalar(tmp8[:], mi8[:], 127, None, OP.bitwise_and)
                nc.vector.tensor_tensor(vglob[:], vglob[:], tmp8[:], OP.add)
                prow = small.tile([128, 1], i32, tag="prow")
                nc.gpsimd.iota(prow[:], pattern=[[0, 1]], base=0, channel_multiplier=128)
                prowu = small.tile([128, 1], u32, tag="prowu")
                nc.vector.tensor_copy(prowu[:], prow[:])
                nc.vector.tensor_tensor(vglob[:], vglob[:], rap(prowu[:], [[0, 8]]), OP.add)
                # own global ids + sc2o = (vg//8+1, vg%8+1)
                vgo = small.tile([128, 8], u32, tag="vgo")
                nc.vector.tensor_tensor(vgo[:], vglob[:], rap(roffo_u[:], [[0, 8]]), OP.add)
                i4o = small.tile([128, 8], u32, tag="i4o")
                nc.vector.tensor_scalar(i4o[:], vgo[:], 3, None, OP.logical_shift_right)
                sbo = small.tile([128, 8], u32, tag="sbo")
                nc.vector.tensor_scalar(sbo[:], vgo[:], 7, None, OP.bitwise_and)
                sc2o = persist.tile([128, 8, 2], f32)
                nc.vector.tensor_copy(rap(sc2o[:], [[2, 8]]), i4o[:])
                nc.vector.tensor_copy(AP(sc2o.tensor, sc2o[:].offset + 1, [list(sc2o[:].ap[0]), [2, 8]]), sbo[:])
                nc.vector.tensor_scalar(sc2o[:], sc2o[:], 1.0, None, OP.add)

                # pair AllGather of [vals | voxel ids]
                nc.vector.tensor_copy(mxpack[:, 8:16], vglob[:])
                nc.gpsimd.dma_start(AP(valin, 0, [[8, 128], [1, 8]]), mxpack[:, 0:8])
                nc.gpsimd.dma_start(AP(valin, 1024, [[8, 128], [1, 8]]), mxpack[:, 8:16])
                NOAG = os.environ.get("KNOAG", "0") == "1" or NOCOLL
                vsrc, voff = (valin, 0) if NOAG else (valout, 2048)
                if not NOAG:
                    nc.gpsimd.collective_compute("AllGather", OP.bypass, ins=[valin[:]], outs=[valout[:]], replica_groups=PAIRS)
                unionAB = persist.tile([128, 2, 1024], f32)
                if NOAG:
                    nc.gpsimd.dma_start(unionAB[:, 0, :], AP(vsrc, 0, [[0, 128], [1, 1024]]))
                    nc.gpsimd.dma_start(unionAB[:, 1, :], AP(vsrc, 0, [[0, 128], [1, 1024]]))
                else:
                    nc.gpsimd.dma_start(unionAB[:], AP(vsrc, 0, [[0, 128], [voff, 2], [1, 1024]]))
                rvb = small.tile([128, 2, 2, 8], f32, tag="rvb")
                if NOAG:
                    nc.gpsimd.dma_start(rvb[:, 0, :, :], AP(vsrc, 0, [[8, 128], [1024, 2], [1, 8]]))
                    nc.gpsimd.dma_start(rvb[:, 1, :, :], AP(vsrc, 0, [[8, 128], [1024, 2], [1, 8]]))
                else:
                    nc.gpsimd.dma_start(rvb[:], AP(vsrc, 0, [[8, 128], [voff, 2], [1024, 2], [1, 8]]))
                rv0 = rvb[:, 0, 0, :]
                rg0 = rvb[:, 0, 1, :]
                rv1 = rvb[:, 1, 0, :]
                rg1 = rvb[:, 1, 1, :]
                # partner candidates: h=0's partner is AG row 1, h=1's is row 0
                pmx8 = persist.tile([128, 8], f32)
                nc.vector.tensor_scalar(pmx8[:], rv0, rH, None, OP.mult)
                nc.vector.scalar_tensor_tensor(pmx8[:], rv1, rHc, pmx8[:], OP.mult, OP.add)
                pvgf = persist.tile([128, 8], f32)
                nc.vector.tensor_scalar(pvgf[:], rg0, rH, None, OP.mult)
                nc.vector.scalar_tensor_tensor(pvgf[:], rg1, rHc, pvgf[:], OP.mult, OP.add)
                # partner sc2
                nc.vector.tensor_tensor(pvgf[:], pvgf[:], rap(roffp_f, [[0, 8]]), OP.add)
                pvgu = small.tile([128, 8], u32, tag="pvgu")
                nc.vector.tensor_copy(pvgu[:], pvgf[:])
                i4p = small.tile([128, 8], u32, tag="i4p")
                nc.vector.tensor_scalar(i4p[:], pvgu[:], 3, None, OP.logical_shift_right)
                sbp = small.tile([128, 8], u32, tag="sbp")
                nc.vector.tensor_scalar(sbp[:], pvgu[:], 7, None, OP.bitwise_and)
                sc2p = persist.tile([128, 8, 2], f32)
                nc.vector.tensor_copy(rap(sc2p[:], [[2, 8]]), i4p[:])
                nc.vector.tensor_copy(AP(sc2p.tensor, sc2p[:].offset + 1, [list(sc2p[:].ap[0]), [2, 8]]), sbp[:])
                nc.vector.tensor_scalar(sc2p[:], sc2p[:], 1.0, None, OP.add)
                # ranks in the 2048-union: one accumulating pass per candidate
                geb2 = persist.tile([128, 2048], f32)
                rkA = small.tile([128, 8], f32, tag="rkA")
                rkB = small.tile([128, 8], f32, tag="rkB")
                uflat = unionAB[:].rearrange('p a b -> p (a b)')
                for c8 in range(8):
                    nc.vector.tensor_scalar(geb2[:], uflat, mxpack[:, c8:c8 + 1], None, OP.is_ge,
                                            OP.add, accum_out=rkA[:, c8:c8 + 1])
                for c8 in range(8):
                    nc.vector.tensor_scalar(geb2[:], uflat, pmx8[:, c8:c8 + 1], None, OP.is_ge,
                                            OP.add, accum_out=rkB[:, c8:c8 + 1])
                slotfo = persist.tile([128, 8], f32)
                nc.vector.tensor_scalar(slotfo[:], rkA[:], -1.0, None, OP.add)
                slotfp = persist.tile([128, 8], f32)
                nc.vector.tensor_scalar(slotfp[:], rkB[:], -1.0, None, OP.add)

            # ---------------- streaming, rest ----------------
            for t in range(TSPLIT, KS2):
                stream_tile(t)

            # ---- chain B1 (mid-stream): scatter matmuls + gather ids ----
            if KHW >= 3:
                ps_sc = psum.tile([128, 2], f32, tag="mm")
                oh16 = persist.tile([128, 16, 128], f32)
                nc.vector.tensor_tensor(oh16[:, 0:8, :],
                                        rap(iota128[:], [[0, 8], [1, 128]]),
                                        rap(slotfo[:], [[1, 8], [0, 128]]), OP.is_equal)
                nc.vector.tensor_tensor(oh16[:, 8:16, :],
                                        rap(iota128[:], [[0, 8], [1, 128]]),
                                        rap(slotfp[:], [[1, 8], [0, 128]]), OP.is_equal)
                for c8 in range(8):
                    nc.tensor.matmul(ps_sc[:], oh16[:, c8, :], sc2o[:, c8, :], start=(c8 == 0), stop=False)
                for c8 in range(8):
                    nc.tensor.matmul(ps_sc[:], oh16[:, 8 + c8, :], sc2p[:, c8, :], start=False, stop=(c8 == 7))
                gslot = persist.tile([128, 2], f32)
                nc.vector.tensor_copy(gslot[:], ps_sc[:])
                subcol = persist.tile([128, 1], f32)
                nc.vector.tensor_scalar(subcol[:], gslot[:, 1:2], -1.0, None, OP.add)
                gidxf = small.tile([128, 1], f32, tag="gidxf")
                nc.vector.tensor_scalar(gidxf[:], gslot[:, 0:1], -1.0, 0.0, OP.add, OP.max)
                nc.vector.tensor_scalar(gidxf[:], gidxf[:], float(V // 8 - 1), None, OP.min)
                gidxi = small.tile([128, 1], i16, tag="gidxi")
                nc.vector.tensor_copy(gidxi[:], gidxf[:])

            for t in range(KS2, KS3):
                stream_tile(t)

            # ---- chain B2 (late-mid-stream): idx round-trip + dma_gather ----
            if KHW >= 4:
                nc.sync.dma_start(AP(idxdr, 0, [[1, 128]]), gidxi[:])
                gidx = small.tile([128, 8], i16, tag="gidx")
                for blk in range(8):
                    qg = nc.sync if blk % 2 == 0 else nc.scalar
                    qg.dma_start(gidx[blk * 16:(blk + 1) * 16, :], AP(idxdr, 0, [[1, 16], [16, 8]]))
                gg = persist.tile([128, 1, GE], f32)
                gin = AP(garr, 0, [[GE, V // 8], [1, GE]])
                nc.gpsimd.dma_gather(gg[:], gin, gidx[:], num_idxs=128, num_idxs_reg=128, elem_size=GE)
                cand = persist.tile([128, GCOLS], f32)
                ohall = small.tile([128, 8], f32, tag="ohl")
                for r in range(8):
                    nc.vector.tensor_scalar(ohall[:, r:r + 1], subcol[:], float(r), None, OP.is_equal)
                nc.vector.tensor_scalar(cand[:], gg[:, 0, 0:GCOLS], ohall[:, 0:1], None, OP.mult)
                for r in range(1, 8):
                    nc.vector.scalar_tensor_tensor(cand[:], gg[:, 0, r * GCOLS:(r + 1) * GCOLS],
                                                   ohall[:, r:r + 1], cand[:], OP.mult, OP.add)
                labm = small.tile([128, 16], f32, tag="labm")
                nc.vector.tensor_tensor(labm[:], cand[:, F:GCOLS], iota16[:], OP.mult)
                labmine = persist.tile([128, 1], f32)
                nc.vector.tensor_reduce(labmine[:], labm[:], AX.X, OP.add)
                labP = labmine[0:K, :]
                heP = cand[0:K, 0:F]
                hePH = persist.tile([K, Fh], f32)
                nc.vector.tensor_scalar(hePH[:], heP[:, 0:Fh], rHc[0:K, :], None, OP.mult)
                nc.vector.scalar_tensor_tensor(hePH[:], heP[:, Fh:F], rH[0:K, :], hePH[:], OP.mult, OP.add)

            for t in range(KS3, KS4):
                stream_tile(t)

            # ---- chain B3 (mid-stream): he/lab transposes + masks ----
            if KHW >= 5:
                ps_hes = psum.tile([Fh, K], f32, tag="mm")
                nc.tensor.transpose(ps_hes[:], hePH[:], ident[0:K, 0:K])
                hh = persist.tile([Fh, K], f32)
                nc.vector.tensor_copy(hh[:], ps_hes[:])
                # he replicated across class partitions via a DRAM round trip
                # (hidden mid-stream / under the sums AllGather)
                nc.gpsimd.dma_start(AP(hehalf, 0, [[K, Fh], [1, K]]), hh[:])
                he_rep = persist.tile([C, Fh * K], f32)
                nc.gpsimd.dma_start(he_rep[:], AP(hehalf, 0, [[0, C], [1, Fh * K]]))
                # lab as a row vector via PE transpose, then one-hot masks via
                # ones-column broadcast matmuls (no DRAM round trips)
                ps_lt = psum.tile([1, 128], f32, tag="mm2")
                nc.tensor.transpose(ps_lt[:], labmine[:], ident[:])
                labrow = persist.tile([1, K], bf16)
                nc.vector.tensor_copy(labrow[:], ps_lt[0:1, 0:K])
                ps_bK = psum.tile([K, K], f32, tag="mm")
                nc.tensor.matmul(ps_bK[:], onesr[:, 0:K], labrow[:], start=True, stop=True)
                mask2 = persist.tile([K, K], f32)
                nc.vector.tensor_scalar(mask2[:], ps_bK[:], labP, None, OP.is_equal)
                ps_b16 = psum.tile([C, K], f32, tag="mm2")
                nc.tensor.matmul(ps_b16[:], onesr[:, 0:C], labrow[:], start=True, stop=True)
                Mp = persist.tile([C, K], f32)
                nc.vector.tensor_tensor(Mp[:], ps_b16[:], iotcf[:], OP.is_equal)
                Mpb = persist.tile([C, K], bf16)
                nc.vector.tensor_copy(Mpb[:], Mp[:])
                Mm1 = persist.tile([C, K], bf16)
                nc.vector.tensor_scalar(Mm1[:], Mp[:], -1.0, None, OP.add)
                M_kc = persist.tile([K, C], f32)
                nc.vector.tensor_scalar(M_kc[:], iota16[0:K, :], labP, None, OP.is_equal)

                nk = small.tile([C, 1], f32, tag="nk")
                nc.vector.tensor_reduce(nk[:], Mp[:], AX.X, OP.add)
                nk2 = small.tile([C, 1], f32, tag="nk2")
                nc.vector.tensor_tensor(nk2[:], nk[:], nk[:], OP.mult)
                den = small.tile([C, 1], f32, tag="den")
                nc.vector.tensor_scalar(den[:], nk2[:], float(F), 1.0, OP.mult, OP.max)
                wc0 = small.tile([C, 1], f32, tag="wc0")
                nc.vector.reciprocal(wc0[:], den[:])
                gnk = small.tile([C, 1], f32, tag="gnk")
                nc.vector.tensor_scalar(gnk[:], nk[:], 0.0, None, OP.is_gt)
                rhs2 = persist.tile([C, 2], f32)
                nc.vector.tensor_tensor(rhs2[:, 0:1], wc0[:], gnk[:], OP.mult)
                nc.vector.tensor_tensor(rhs2[:, 1:2], rhs2[:, 0:1], nk[:], OP.mult)

            for t in range(KS4, NT):
                stream_tile(t)

            # ---------------- sums epilogue + AllGather ----------------
            sums_sb = persist.tile([C, F + 1], f32)
            nc.vector.tensor_copy(sums_sb[:], ps_sums[:])
            nc.sync.dma_start(AP(sumin, 0, [[F, C], [1, F]]), sums_sb[:, 0:F])
            nc.sync.dma_start(AP(sumin, 1024, [[1, C]]), sums_sb[:, F:F + 1])
            if NOCOLL:
                nc.gpsimd.dma_start(AP(sumout, 0, [[SUMN, 8], [1, SUMN]]),
                                    AP(sumin, 0, [[0, 8], [1, SUMN]]))
            else:
                nc.gpsimd.collective_compute("AllGather", OP.bypass, ins=[sumin[:]], outs=[sumout[:]], replica_groups=ALL)

            # ---------------- post-AG tail ----------------
            if KHW >= 6:
                # local reduce of the gathered per-core sums/counts
                tot8 = persist.tile([C, F, 8], f32)
                nc.sync.dma_start(tot8[:], AP(sumout, 0, [[F, C], [1, F], [SUMN, 8]]))
                totc8 = small.tile([C, 8], f32, tag="totc8")
                nc.scalar.dma_start(totc8[:], AP(sumout, 1024, [[1, C], [SUMN, 8]]))
                tot = persist.tile([C, F], f32)
                nc.vector.tensor_reduce(tot[:], tot8[:], AX.X, OP.add)
                totc = small.tile([C, 1], f32, tag="totc")
                nc.vector.tensor_reduce(totc[:], totc8[:], AX.X, OP.add)
                cmax = small.tile([C, 1], f32, tag="cmax")
                nc.vector.tensor_scalar(cmax[:], totc[:], 1.0, None, OP.max)
                crec = small.tile([C, 1], f32, tag="crec")
                nc.vector.reciprocal(crec[:], cmax[:])
                cgt = small.tile([C, 1], f32, tag="cgt")
                nc.vector.tensor_scalar(cgt[:], totc[:], 0.0, None, OP.is_gt)
                csc = small.tile([C, 1], f32, tag="csc")
                nc.vector.tensor_scalar(csc[:], crec[:], cgt[:], THETA, OP.mult, OP.mult)
                avg = persist.tile([C, F], f32)
                nc.vector.tensor_scalar(avg[:], tot[:], csc[:], None, OP.mult)
                avgH = persist.tile([C, Fh], f32)
                nc.vector.tensor_scalar(avgH[:], avg[:, 0:Fh], rHc[0:C, :], None, OP.mult)
                nc.vector.scalar_tensor_tensor(avgH[:], avg[:, Fh:F], rH[0:C, :], avgH[:], OP.mult, OP.add)
                avgHb = persist.tile([C, Fh], bf16)
                nc.vector.tensor_copy(avgHb[:], avgH[:])
                ps_avt = psum.tile([Fh, C], f32, tag="mm")
                nc.tensor.transpose(ps_avt[:], avgH[:], ident[0:C, 0:C])
                avgHT = persist.tile([Fh, C], f32)
                nc.vector.tensor_copy(avgHT[:], ps_avt[:])

                # Ei[i, f] = exp(he_i[f] * avg[lab_i, f] / tau)  as [K, Fh]
                ps_a = psum.tile([Fh, K], f32, tag="mm2")
                nc.tensor.matmul(ps_a[:], avgHb[:], Mpb[:], start=True, stop=True)
                prodT = persist.tile([Fh, K], f32)
                nc.vector.tensor_tensor(prodT[:], hh[:], ps_a[:], OP.mult)
                ps_pt = psum.tile([K, Fh], f32, tag="mm")
                nc.tensor.transpose(ps_pt[:], prodT[:], ident[0:Fh, 0:Fh])
                Ei = persist.tile([K, Fh], f32)
                nc.scalar.activation(Ei[:], ps_pt[:], AF.Exp, scale=1.0 / TAU)

                # pair term, two phases to avoid ACT table thrash:
                #   phase A (all Exp): Ep[c, f*K+j] = exp(avg[c,f] he[f,j] / tau)
                #   phase B (all Ln):  neg = -(Mm1^T @ Ep);  term = ln(Ei + neg)
                Ep = persist.tile([C, Fh * K], bf16)
                for q in range(NQ):
                    sl = slice(q * FQ * K, (q + 1) * FQ * K)
                    nc.vector.tensor_tensor(
                        rap(Ep[:, sl], [[K, FQ], [1, K]]),
                        rap(he_rep[:, sl], [[K, FQ], [1, K]]),
                        rap(avgH[:, q * FQ:(q + 1) * FQ], [[1, FQ], [0, K]]), OP.mult)
                    nc.scalar.activation(Ep[:, sl], Ep[:, sl], AF.Exp, scale=1.0 / TAU)
                S2cols = persist.tile([K, NQ], f32)
                for q in range(NQ):
                    ps_pr = psx.tile([K, FQ * K], f32, tag="pr")
                    # psum-bank limit: a single matmul's out must stay in one
                    # 512-f32 bank, so split the 800-col chunk at 512
                    for lo, hi in ((0, 512), (512, FQ * K)):
                        nc.tensor.matmul(ps_pr[:, lo:hi], Mm1[:],
                                         Ep[:, q * FQ * K + lo:q * FQ * K + hi],
                                         start=True, stop=True)
                    termin = small.tile([K, FQ * K], f32, tag="termin")
                    nc.vector.tensor_tensor(rap(termin[:], [[K, FQ], [1, K]]),
                                            rap(Ei[:, q * FQ:(q + 1) * FQ], [[1, FQ], [0, K]]),
                                            rap(ps_pr[:], [[K, FQ], [1, K]]), OP.subtract)
                    termf = small.tile([K, FQ * K], f32, tag="termf")
                    nc.scalar.activation(termf[:], termin[:], AF.Ln)
                    scr = small.tile([K, FQ * K], f32, tag="scr")
                    nc.vector.tensor_tensor(rap(scr[:], [[K, FQ], [1, K]]),
                                            rap(mask2[:], [[0, FQ], [1, K]]),
                                            rap(termf[:], [[K, FQ], [1, K]]), OP.mult)
                    nc.vector.tensor_reduce(S2cols[:, q:q + 1], scr[:], AX.X, OP.add)
                S2 = small.tile([K, 1], f32, tag="S2")
                nc.vector.tensor_reduce(S2[:], S2cols[:], AX.X, OP.add)

                ps_u = psum.tile([K, 2], f32, tag="mm")
                nc.tensor.matmul(ps_u[:], Mp[:], rhs2[:], start=True, stop=True)
                U = small.tile([K, 2], f32, tag="U")
                nc.vector.tensor_copy(U[:], ps_u[:])
                ps_g = psum.tile([K, C], f32, tag="mm2")
                nc.tensor.matmul(ps_g[:], hh[:], avgHT[:], start=True, stop=True)
                Gm = small.tile([K, C], f32, tag="Gm")
                nc.vector.tensor_tensor(Gm[:], ps_g[:], M_kc[:], OP.mult)
                li = small.tile([K, 1], f32, tag="li")
                nc.vector.tensor_reduce(li[:], Gm[:], AX.X, OP.add)

                t1 = small.tile([K, 1], f32, tag="t1")
                nc.vector.tensor_tensor(t1[:], S2[:], U[:, 0:1], OP.mult)
                t2 = small.tile([K, 1], f32, tag="t2")
                nc.vector.tensor_tensor(t2[:], li[:], U[:, 1:2], OP.mult)
                cvec = small.tile([K, 1], f32, tag="cvec")
                nc.vector.scalar_tensor_tensor(cvec[:], t2[:], -1.0 / TAU, t1[:], OP.mult, OP.add)

                ps_t = psum.tile([1, 1], f32, tag="mm")
                nc.tensor.matmul(ps_t[:], cvec[:], ones128[0:K, :], start=True, stop=True)
                lossp = small.tile([1, 8], f32, tag="lossp")
                nc.vector.memset(lossp[:], 0.0)
                nc.vector.tensor_scalar(lossp[:, 0:1], ps_t[:], -1.0 / B, None, OP.mult)
                nc.sync.dma_start(lin[:], lossp[:])
                if NOCOLL:
                    nc.gpsimd.dma_start(AP(lout, 0, [[8, 8], [1, 8]]),
                                        AP(lin, 0, [[0, 8], [1, 8]]))
                else:
                    nc.gpsimd.collective_compute("AllGather", OP.bypass, ins=[lin[:]], outs=[lout[:]], replica_groups=ALL)
                res8 = small.tile([1, 8], f32, tag="res8")
                nc.sync.dma_start(res8[:], AP(lout, 0, [[0, 1], [8, 8]]))
                res = small.tile([1, 1], f32, tag="res")
                nc.vector.tensor_reduce(res[:], res8[:], AX.X, OP.add)
                nc.sync.dma_start(out[:], res[:])
            else:
                resd = small.tile([1, 1], f32, tag="resd")
                nc.sync.dma_start(resd[:], AP(sumout, 0, [[1, 1], [1, 1]]))
                nc.sync.dma_start(out[:], resd[:])

    nc.compile()
    ctx.close()
    return nc


def make_in_maps(proba, y, embeddings):
    import ml_dtypes
    bf = ml_dtypes.bfloat16
    in_maps = []
    for core in range(NCORES):
        b, h = core // 2, core % 2
        sl = slice(h * Vh, (h + 1) * Vh)
        pT = np.ascontiguousarray(proba[b, :, sl].T.astype(bf))
        yT = np.ascontiguousarray(y[b, :, sl].T.astype(bf))
        eT = np.ascontiguousarray(
            np.concatenate([embeddings[b, :, sl].T,
                            np.ones((Vh, 1), np.float32)], axis=1).astype(bf))
        ga = np.ascontiguousarray(
            np.concatenate([embeddings[b].T, y[b].T], axis=1).astype(np.float32))
        rl = np.array([[h, 1 - h, h * Vh, (1 - h) * Vh, 0, 0, 0, 0]], dtype=np.float32)
        in_maps.append({"pt": pT, "yt": yT, "et": eT, "garr": ga, "role": rl})
    return in_maps


_NC = None


def kernel(proba, y, embeddings):
    global _NC
    from concourse.bass_utils import run_bass_kernel_spmd

    if _NC is None:
        _NC = build_program()
    in_maps = make_in_maps(proba, y, embeddings)
    res = run_bass_kernel_spmd(_NC, in_maps, core_ids=list(range(NCORES)))
    return np.float32(res.results[0]["out"].reshape(())).reshape(())


# revision 47
# speedup vs baseline: 3.7588x; 1.6293x over previous
"""AnatomicalContrastiveLoss on 8 trn2 NeuronCores (Bass/Tile), v3.

Sharding: core c = (b = c//2, h = c%2) streams its half-volume shard
(probaT f32 / yT bf16 / embT bf16, voxel-major; e/y host-cast to bf16 to
halve HBM traffic) computing
  - ps_sums[c, f|count] = sum_v y[v,c] * e65[v,f]  (1024 bf16 PE matmuls;
    e65 carries a ones column so counts ride in psum col 64)
  - w[v] = sum_c ln(p[v,c] + 1e-30)  (monotonic stand-in for prod_c p)

Top-k: per-partition-row top-8 of w gives 1024 candidates per half; ONE pair
AllGather shares candidate values AND voxel ids (8KB). Both cores rank the
2048-value union locally, scatter all 2048 candidates' (gather-id, sub-row)
into a [128,2] psum via one-hot matmuls, and dma_gather the top-100 rows
from a FULL-volume [e|y] gather array (identical on both pair cores).

Tail: f-range split across the pair (h=0 -> f 0:32, h=1 -> f 32:64) with
role-blended slices. Class sums/counts exchanged via AllGather + local
reduce (fewer ring steps than AllReduce). Broadcasts built on-chip via
ones-column matmuls (no DRAM round trips except he_rep, hidden mid-stream).
Pair term in NQ chunks: matmul (mask-1)^T @ Ep = -neg, DVE subtract with Ei
broadcast, one Ln per chunk, masked reduce. Final scalar via AllGather +
local sum.
"""
import os
from contextlib import ExitStack

import numpy as np

B, C, F = 4, 16, 64
Fh = F // 2            # per-core f-range in the tail
V = 262144
Vh = V // 2            # per-core voxels
K = 100
TAU, THETA = 0.1, 0.9
GCOLS = 80             # gather row: 64 emb + 16 y
GE = 8 * GCOLS         # gather element: 8 voxel-rows = 640 f32 = 2560B
SUMN = 1040            # sums 1024 (c*64+f) | count 16
NCORES = 8
NT = 16                # stream tiles
TV = Vh // NT
NS = TV // 128
NTP = 8                # pass1 (proba) chunks
TVP = Vh // NTP
NSP = TVP // 128
NQ = 4                 # tail chunks (8 f-slices each)
FQ = Fh // NQ


def build_program(stage=None):
    from concourse import bass, bacc, tile, mybir, masks

    f32 = mybir.dt.float32
    bf16 = mybir.dt.bfloat16
    u32 = mybir.dt.uint32
    i32 = mybir.dt.int32
    i16 = mybir.dt.int16
    AF = mybir.ActivationFunctionType
    OP = mybir.AluOpType
    AX = mybir.AxisListType
    AP = bass.AP

    def rap(base, free_dims):
        return AP(base.tensor, base.offset, [list(base.ap[0])] + [list(d) for d in free_dims])

    TSPLIT = int(os.environ.get("KSPLIT", "8"))
    KS2 = int(os.environ.get("KSPLIT2", "10"))
    KS3 = int(os.environ.get("KSPLIT3", "11"))
    KS4 = int(os.environ.get("KSPLIT4", "13"))
    NREP = int(os.environ.get("KREPEAT", "1"))
    NOCOLL = os.environ.get("KNOCOLL", "0") == "1"
    KHW = int(os.environ.get("KHW", "6"))

    nc = bacc.Bacc(None, target_bir_lowering=False)
    ctx = ExitStack()

    pt = nc.dram_tensor("pt", [Vh, C], bf16, kind="ExternalInput")
    yt = nc.dram_tensor("yt", [Vh, C], bf16, kind="ExternalInput")
    et = nc.dram_tensor("et", [Vh, F + 1], bf16, kind="ExternalInput")
    garr = nc.dram_tensor("garr", [V, GCOLS], f32, kind="ExternalInput")
    role = nc.dram_tensor("role", [1, 8], f32, kind="ExternalInput")
    out = nc.dram_tensor("out", [1, 1], f32, kind="ExternalOutput")

    idxdr = nc.dram_tensor("idxdr", [128], i16)
    hehalf = nc.dram_tensor("hehalf", [Fh * K], f32)
    sumin = nc.dram_tensor("sumin", [1, SUMN], f32)
    sumout = nc.dram_tensor("sumout", [8, SUMN], f32)
    valin = nc.dram_tensor("valin", [1, 2048], f32)
    valout = nc.dram_tensor("valout", [2, 2048], f32)
    lin = nc.dram_tensor("lin", [1, 8], f32)
    lout = nc.dram_tensor("lout", [8, 8], f32)

    ALL = [list(range(NCORES))]
    PAIRS = [[2 * i, 2 * i + 1] for i in range(B)]

    with tile.TileContext(nc) as tc:
        with (
            tc.tile_pool(name="stream", bufs=4) as stream,
            tc.tile_pool(name="pass1", bufs=8) as pass1,
            tc.tile_pool(name="persist", bufs=1) as persist,
            tc.tile_pool(name="small", bufs=2) as small,
            tc.tile_pool(name="psum", bufs=1, space="PSUM") as psum,
            tc.tile_pool(name="psx", bufs=2, space="PSUM") as psx,
        ):
          for _rep in range(NREP):
            ident = persist.tile([128, 128], f32)
            masks.make_identity(nc, ident[:])
            ones128 = persist.tile([128, 1], f32)
            nc.vector.memset(ones128[:], 1.0)
            onesr = persist.tile([1, 128], bf16)
            nc.vector.memset(onesr[:], 1.0)
            iota128 = persist.tile([128, 128], f32)
            iot1 = small.tile([128, 128], i32, tag="iot1")
            nc.gpsimd.iota(iot1[:], pattern=[[1, 128]], base=0, channel_multiplier=0)
            nc.vector.tensor_copy(iota128[:], iot1[:])
            iota16 = persist.tile([128, 16], f32)
            ioti = small.tile([128, 16], i32, tag="ioti")
            nc.gpsimd.iota(ioti[:], pattern=[[1, 16]], base=0, channel_multiplier=0)
            nc.vector.tensor_copy(iota16[:], ioti[:])
            iotc = small.tile([C, K], i32, tag="iotc")
            nc.gpsimd.iota(iotc[:], pattern=[[0, K]], base=0, channel_multiplier=1)
            iotcf = persist.tile([C, K], f32)
            nc.vector.tensor_copy(iotcf[:], iotc[:])
            # role scalars: [h, 1-h, own_off, partner_off] broadcast to partitions
            rtile = persist.tile([128, 8], f32)
            nc.sync.dma_start(rtile[:], AP(role, 0, [[0, 128], [1, 8]]))
            rH = rtile[:, 0:1]
            rHc = rtile[:, 1:2]
            roffo_u = persist.tile([128, 1], u32)
            nc.vector.tensor_copy(roffo_u[:], rtile[:, 2:3])
            roffp_f = rtile[:, 3:4]
            # preload the Exp activation table while ACT is idle
            actwarm = persist.tile([1, 1], f32)
            nc.scalar.activation(actwarm[:], ones128[0:1, :], AF.Exp)

            # ---------------- streaming ----------------
            lnbias = persist.tile([128, 1], f32)
            nc.vector.memset(lnbias[:], 1e-30)
            w_sb = persist.tile([128, NTP, NSP], f32)
            ptiles = []
            for t in range(NTP):
                ptile = pass1.tile([128, NSP, C], bf16, tag="p")
                qp = nc.sync if t % 2 == 0 else nc.scalar
                qp.dma_start(ptile[:], AP(pt, t * TVP * C, [[NSP * C, 128], [C, NSP], [1, C]]))
                ptiles.append(ptile)

            def w_chunk(t):
                # w = sum_c ln(p + 1e-30): monotonic stand-in for prod_c p
                # (p is bf16; Ln output must stay f32 for ranking precision)
                pf = ptiles[t][:].rearrange('p a b -> p (a b)')
                lnp = pass1.tile([128, NSP * C], f32, tag="lnp", bufs=2)
                nc.scalar.activation(lnp[:], pf, AF.Ln, bias=lnbias[:])
                nc.vector.tensor_reduce(w_sb[:, t, :], rap(lnp[:], [[C, NSP], [1, C]]), AX.X, OP.add)

            ps_sums = psum.tile([C, F + 1], f32, tag="sums")
            def stream_tile(t):
                ytile = stream.tile([128, NS, C], bf16, tag="y")
                etile = stream.tile([128, NS, F + 1], bf16, tag="e")
                qy = nc.scalar if t % 2 == 0 else nc.sync
                qy.dma_start(ytile[:], AP(yt, t * TV * C, [[NS * C, 128], [C, NS], [1, C]]))
                q = nc.sync if t % 2 == 0 else nc.scalar
                q.dma_start(etile[:], AP(et, t * TV * (F + 1), [[NS * (F + 1), 128], [F + 1, NS], [1, F + 1]]))
                for s in range(NS):
                    nc.tensor.matmul(
                        ps_sums[:], ytile[:, s, :], etile[:, s, :],
                        start=(t == 0 and s == 0), stop=(t == NT - 1 and s == NS - 1),
                    )

            for t in range(TSPLIT):
                stream_tile(t)
                if t < NTP:
                    w_chunk(t)
            for t in range(TSPLIT, NTP):
                w_chunk(t)

            # ---------------- chain A: top-k candidates + pair AG + ranks ----
            _pr = tc.cur_priority
            tc.cur_priority = 0
            if KHW >= 2:
                mxpack = persist.tile([128, 16], f32)
                mx8 = mxpack[:, 0:8]
                nc.vector.max(mx8, w_sb[:].rearrange('p a b -> p (a b)'))
                mi8 = persist.tile([128, 8], u32)
                nc.vector.max_index(mi8[:], mx8, w_sb[:].rearrange('p a b -> p (a b)'))
                # v_local = ((mi>>7)<<14) + p*128 + (mi&127)
                vglob = persist.tile([128, 8], u32)
                nc.vector.tensor_scalar(vglob[:], mi8[:], 7, None, OP.logical_shift_right)
                nc.vector.tensor_scalar(vglob[:], vglob[:], 14, None, OP.logical_shift_left)
                tmp8 = small.tile([128, 8], u32, tag="tmp8")
                nc.vector.tensor_scalar(tmp8[:], mi8[:], 127, None, OP.bitwise_and)
                nc.vector.tensor_tensor(vglob[:], vglob[:], tmp8[:], OP.add)
                prow = small.tile([128, 1], i32, tag="prow")
                nc.gpsimd.iota(prow[:], pattern=[[0, 1]], base=0, channel_multiplier=128)
                prowu = small.tile([128, 1], u32, tag="prowu")
                nc.vector.tensor_copy(prowu[:], prow[:])
                nc.vector.tensor_tensor(vglob[:], vglob[:], rap(prowu[:], [[0, 8]]), OP.add)
                # own global ids + sc2o = (vg//8+1, vg%8+1)
                vgo = small.tile([128, 8], u32, tag="vgo")
                nc.vector.tensor_tensor(vgo[:], vglob[:], rap(roffo_u[:], [[0, 8]]), OP.add)
                i4o = small.tile([128, 8], u32, tag="i4o")
                nc.vector.tensor_scalar(i4o[:], vgo[:], 3, None, OP.logical_shift_right)
                sbo = small.tile([128, 8], u32, tag="sbo")
                nc.vector.tensor_scalar(sbo[:], vgo[:], 7, None, OP.bitwise_and)
                sc2o = persist.tile([128, 8, 2], f32)
                nc.vector.tensor_copy(rap(sc2o[:], [[2, 8]]), i4o[:])
                nc.vector.tensor_copy(AP(sc2o.tensor, sc2o[:].offset + 1, [list(sc2o[:].ap[0]), [2, 8]]), sbo[:])
                nc.vector.tensor_scalar(sc2o[:], sc2o[:], 1.0, None, OP.add)

                # pair AllGather of [vals | voxel ids]
                nc.vector.tensor_copy(mxpack[:, 8:16], vglob[:])
                nc.gpsimd.dma_start(AP(valin, 0, [[8, 128], [1, 8]]), mxpack[:, 0:8])
                nc.gpsimd.dma_start(AP(valin, 1024, [[8, 128], [1, 8]]), mxpack[:, 8:16])
                NOAG = os.environ.get("KNOAG", "0") == "1" or NOCOLL
                vsrc, voff = (valin, 0) if NOAG else (valout, 2048)
                if not NOAG:
                    nc.gpsimd.collective_compute("AllGather", OP.bypass, ins=[valin[:]], outs=[valout[:]], replica_groups=PAIRS)
                unionAB = persist.tile([128, 2, 1024], f32)
                if NOAG:
                    nc.gpsimd.dma_start(unionAB[:, 0, :], AP(vsrc, 0, [[0, 128], [1, 1024]]))
                    nc.gpsimd.dma_start(unionAB[:, 1, :], AP(vsrc, 0, [[0, 128], [1, 1024]]))
                else:
                    nc.gpsimd.dma_start(unionAB[:], AP(vsrc, 0, [[0, 128], [voff, 2], [1, 1024]]))
                rvb = small.tile([128, 2, 2, 8], f32, tag="rvb")
                if NOAG:
                    nc.gpsimd.dma_start(rvb[:, 0, :, :], AP(vsrc, 0, [[8, 128], [1024, 2], [1, 8]]))
                    nc.gpsimd.dma_start(rvb[:, 1, :, :], AP(vsrc, 0, [[8, 128], [1024, 2], [1, 8]]))
                else:
                    nc.gpsimd.dma_start(rvb[:], AP(vsrc, 0, [[8, 128], [voff, 2], [1024, 2], [1, 8]]))
                rv0 = rvb[:, 0, 0, :]
                rg0 = rvb[:, 0, 1, :]
                rv1 = rvb[:, 1, 0, :]
                rg1 = rvb[:, 1, 1, :]
                # partner candidates: h=0's partner is AG row 1, h=1's is row 0
                pmx8 = persist.tile([128, 8], f32)
                nc.vector.tensor_scalar(pmx8[:], rv0, rH, None, OP.mult)
                nc.vector.scalar_tensor_tensor(pmx8[:], rv1, rHc, pmx8[:], OP.mult, OP.add)
                pvgf = persist.tile([128, 8], f32)
                nc.vector.tensor_scalar(pvgf[:], rg0, rH, None, OP.mult)
                nc.vector.scalar_tensor_tensor(pvgf[:], rg1, rHc, pvgf[:], OP.mult, OP.add)
                # partner sc2
                nc.vector.tensor_tensor(pvgf[:], pvgf[:], rap(roffp_f, [[0, 8]]), OP.add)
                pvgu = small.tile([128, 8], u32, tag="pvgu")
                nc.vector.tensor_copy(pvgu[:], pvgf[:])
                i4p = small.tile([128, 8], u32, tag="i4p")
                nc.vector.tensor_scalar(i4p[:], pvgu[:], 3, None, OP.logical_shift_right)
                sbp = small.tile([128, 8], u32, tag="sbp")
                nc.vector.tensor_scalar(sbp[:], pvgu[:], 7, None, OP.bitwise_and)
                sc2p = persist.tile([128, 8, 2], f32)
                nc.vector.tensor_copy(rap(sc2p[:], [[2, 8]]), i4p[:])
                nc.vector.tensor_copy(AP(sc2p.tensor, sc2p[:].offset + 1, [list(sc2p[:].ap[0]), [2, 8]]), sbp[:])
                nc.vector.tensor_scalar(sc2p[:], sc2p[:], 1.0, None, OP.add)
                # ranks in the 2048-union: one accumulating pass per candidate
                geb2 = persist.tile([128, 2048], f32)
                rkA = small.tile([128, 8], f32, tag="rkA")
                rkB = small.tile([128, 8], f32, tag="rkB")
                uflat = unionAB[:].rearrange('p a b -> p (a b)')
                for c8 in range(8):
                    nc.vector.tensor_scalar(geb2[:], uflat, mxpack[:, c8:c8 + 1], None, OP.is_ge,
                                            OP.add, accum_out=rkA[:, c8:c8 + 1])
                for c8 in range(8):
                    nc.vector.tensor_scalar(geb2[:], uflat, pmx8[:, c8:c8 + 1], None, OP.is_ge,
                                            OP.add, accum_out=rkB[:, c8:c8 + 1])
                slotfo = persist.tile([128, 8], f32)
                nc.vector.tensor_scalar(slotfo[:], rkA[:], -1.0, None, OP.add)
                slotfp = persist.tile([128, 8], f32)
                nc.vector.tensor_scalar(slotfp[:], rkB[:], -1.0, None, OP.add)

            tc.cur_priority = _pr

            # ---------------- streaming, rest ----------------
            for t in range(TSPLIT, KS2):
                stream_tile(t)

            # ---- chain B1 (mid-stream): scatter matmuls + gather ids ----
            _pr = tc.cur_priority
            tc.cur_priority = 0
            if KHW >= 3:
                ps_sc = psum.tile([128, 2], f32, tag="mm")
                oh16 = persist.tile([128, 16, 128], f32)
                nc.vector.tensor_tensor(oh16[:, 0:8, :],
                                        rap(iota128[:], [[0, 8], [1, 128]]),
                                        rap(slotfo[:], [[1, 8], [0, 128]]), OP.is_equal)
                nc.vector.tensor_tensor(oh16[:, 8:16, :],
                                        rap(iota128[:], [[0, 8], [1, 128]]),
                                        rap(slotfp[:], [[1, 8], [0, 128]]), OP.is_equal)
                for c8 in range(8):
                    nc.tensor.matmul(ps_sc[:], oh16[:, c8, :], sc2o[:, c8, :], start=(c8 == 0), stop=False)
                for c8 in range(8):
                    nc.tensor.matmul(ps_sc[:], oh16[:, 8 + c8, :], sc2p[:, c8, :], start=False, stop=(c8 == 7))
                gslot = persist.tile([128, 2], f32)
                nc.vector.tensor_copy(gslot[:], ps_sc[:])
                subcol = persist.tile([128, 1], f32)
                nc.vector.tensor_scalar(subcol[:], gslot[:, 1:2], -1.0, None, OP.add)
                gidxf = small.tile([128, 1], f32, tag="gidxf")
                nc.vector.tensor_scalar(gidxf[:], gslot[:, 0:1], -1.0, 0.0, OP.add, OP.max)
                nc.vector.tensor_scalar(gidxf[:], gidxf[:], float(V // 8 - 1), None, OP.min)
                gidxi = small.tile([128, 1], i16, tag="gidxi")
                nc.vector.tensor_copy(gidxi[:], gidxf[:])

            tc.cur_priority = _pr
            for t in range(KS2, KS3):
                stream_tile(t)

            # ---- chain B2 (late-mid-stream): idx round-trip + dma_gather ----
            _pr = tc.cur_priority
            tc.cur_priority = 0
            if KHW >= 4:
                nc.sync.dma_start(AP(idxdr, 0, [[1, 128]]), gidxi[:])
                gidx = small.tile([128, 8], i16, tag="gidx")
                for blk in range(8):
                    qg = nc.sync if blk % 2 == 0 else nc.scalar
                    qg.dma_start(gidx[blk * 16:(blk + 1) * 16, :], AP(idxdr, 0, [[1, 16], [16, 8]]))
                gg = persist.tile([128, 1, GE], f32)
                gin = AP(garr, 0, [[GE, V // 8], [1, GE]])
                nc.gpsimd.dma_gather(gg[:], gin, gidx[:], num_idxs=128, num_idxs_reg=128, elem_size=GE)
                cand = persist.tile([128, GCOLS], f32)
                ohall = small.tile([128, 8], f32, tag="ohl")
                for r in range(8):
                    nc.vector.tensor_scalar(ohall[:, r:r + 1], subcol[:], float(r), None, OP.is_equal)
                nc.vector.tensor_scalar(cand[:], gg[:, 0, 0:GCOLS], ohall[:, 0:1], None, OP.mult)
                for r in range(1, 8):
                    nc.vector.scalar_tensor_tensor(cand[:], gg[:, 0, r * GCOLS:(r + 1) * GCOLS],
                                                   ohall[:, r:r + 1], cand[:], OP.mult, OP.add)
                labm = small.tile([128, 16], f32, tag="labm")
                nc.vector.tensor_tensor(labm[:], cand[:, F:GCOLS], iota16[:], OP.mult)
                labmine = persist.tile([128, 1], f32)
                nc.vector.tensor_reduce(labmine[:], labm[:], AX.X, OP.add)
                labP = labmine[0:K, :]
                heP = cand[0:K, 0:F]
                hePH = persist.tile([K, Fh], f32)
                nc.vector.tensor_scalar(hePH[:], heP[:, 0:Fh], rHc[0:K, :], None, OP.mult)
                nc.vector.scalar_tensor_tensor(hePH[:], heP[:, Fh:F], rH[0:K, :], hePH[:], OP.mult, OP.add)

            tc.cur_priority = _pr
            for t in range(KS3, KS4):
                stream_tile(t)

            # ---- chain B3 (mid-stream): he/lab transposes + masks ----
            _pr = tc.cur_priority
            tc.cur_priority = 0
            if KHW >= 5:
                ps_hes = psum.tile([Fh, K], f32, tag="mm")
                nc.tensor.transpose(ps_hes[:], hePH[:], ident[0:K, 0:K])
                hh = persist.tile([Fh, K], f32)
                nc.vector.tensor_copy(hh[:], ps_hes[:])
                # he replicated across class partitions via a DRAM round trip
                # (hidden mid-stream / under the sums AllGather)
                nc.gpsimd.dma_start(AP(hehalf, 0, [[K, Fh], [1, K]]), hh[:])
                he_rep = persist.tile([C, Fh * K], f32)
                nc.gpsimd.dma_start(he_rep[:], AP(hehalf, 0, [[0, C], [1, Fh * K]]))
                # lab as a row vector via PE transpose, then one-hot masks via
                # ones-column broadcast matmuls (no DRAM round trips)
                ps_lt = psum.tile([1, 128], f32, tag="mm2")
                nc.tensor.transpose(ps_lt[:], labmine[:], ident[:])
                labrow = persist.tile([1, K], bf16)
                nc.vector.tensor_copy(labrow[:], ps_lt[0:1, 0:K])
                ps_bK = psum.tile([K, K], f32, tag="mm")
                nc.tensor.matmul(ps_bK[:], onesr[:, 0:K], labrow[:], start=True, stop=True)
                mask2 = persist.tile([K, K], f32)
                nc.vector.tensor_scalar(mask2[:], ps_bK[:], labP, None, OP.is_equal)
                ps_b16 = psum.tile([C, K], f32, tag="mm2")
                nc.tensor.matmul(ps_b16[:], onesr[:, 0:C], labrow[:], start=True, stop=True)
                Mp = persist.tile([C, K], f32)
                nc.vector.tensor_tensor(Mp[:], ps_b16[:], iotcf[:], OP.is_equal)
                Mpb = persist.tile([C, K], bf16)
                nc.vector.tensor_copy(Mpb[:], Mp[:])
                Mm1 = persist.tile([C, K], bf16)
                nc.vector.tensor_scalar(Mm1[:], Mp[:], -1.0, None, OP.add)
                M_kc = persist.tile([K, C], f32)
                nc.vector.tensor_scalar(M_kc[:], iota16[0:K, :], labP, None, OP.is_equal)

                nk = small.tile([C, 1], f32, tag="nk")
                nc.vector.tensor_reduce(nk[:], Mp[:], AX.X, OP.add)
                nk2 = small.tile([C, 1], f32, tag="nk2")
                nc.vector.tensor_tensor(nk2[:], nk[:], nk[:], OP.mult)
                den = small.tile([C, 1], f32, tag="den")
                nc.vector.tensor_scalar(den[:], nk2[:], float(F), 1.0, OP.mult, OP.max)
                wc0 = small.tile([C, 1], f32, tag="wc0")
                nc.vector.reciprocal(wc0[:], den[:])
                gnk = small.tile([C, 1], f32, tag="gnk")
                nc.vector.tensor_scalar(gnk[:], nk[:], 0.0, None, OP.is_gt)
                rhs2 = persist.tile([C, 2], f32)
                nc.vector.tensor_tensor(rhs2[:, 0:1], wc0[:], gnk[:], OP.mult)
                nc.vector.tensor_tensor(rhs2[:, 1:2], rhs2[:, 0:1], nk[:], OP.mult)

            tc.cur_priority = _pr
            for t in range(KS4, NT):
                stream_tile(t)

            # ---------------- sums epilogue + AllGather ----------------
            # on ACT, not DVE: this op waits for the whole stream (S[PE] full
            # count); on DVE it head-of-line blocks the rank/gather chain.
            # Copy/identity is in every ACT table set, so no table reload.
            sums_sb = persist.tile([C, F + 1], f32)
            nc.scalar.activation(sums_sb[:], ps_sums[:], AF.Copy)
            nc.sync.dma_start(AP(sumin, 0, [[F, C], [1, F]]), sums_sb[:, 0:F])
            nc.sync.dma_start(AP(sumin, 1024, [[1, C]]), sums_sb[:, F:F + 1])
            if NOCOLL:
                nc.gpsimd.dma_start(AP(sumout, 0, [[SUMN, 8], [1, SUMN]]),
                                    AP(sumin, 0, [[0, 8], [1, SUMN]]))
            else:
                nc.gpsimd.collective_compute("AllGather", OP.bypass, ins=[sumin[:]], outs=[sumout[:]], replica_groups=ALL)

            # ---------------- post-AG tail ----------------
            if KHW >= 6:
                # local reduce of the gathered per-core sums/counts
                tot8 = persist.tile([C, F, 8], f32)
                nc.sync.dma_start(tot8[:], AP(sumout, 0, [[F, C], [1, F], [SUMN, 8]]))
                totc8 = small.tile([C, 8], f32, tag="totc8")
                nc.scalar.dma_start(totc8[:], AP(sumout, 1024, [[1, C], [SUMN, 8]]))
                tot = persist.tile([C, F], f32)
                nc.vector.tensor_reduce(tot[:], tot8[:], AX.X, OP.add)
                totc = small.tile([C, 1], f32, tag="totc")
                nc.vector.tensor_reduce(totc[:], totc8[:], AX.X, OP.add)
                cmax = small.tile([C, 1], f32, tag="cmax")
                nc.vector.tensor_scalar(cmax[:], totc[:], 1.0, None, OP.max)
                crec = small.tile([C, 1], f32, tag="crec")
                nc.vector.reciprocal(crec[:], cmax[:])
                cgt = small.tile([C, 1], f32, tag="cgt")
                nc.vector.tensor_scalar(cgt[:], totc[:], 0.0, None, OP.is_gt)
                csc = small.tile([C, 1], f32, tag="csc")
                nc.vector.tensor_scalar(csc[:], crec[:], cgt[:], THETA, OP.mult, OP.mult)
                avg = persist.tile([C, F], f32)
                nc.vector.tensor_scalar(avg[:], tot[:], csc[:], None, OP.mult)
                avgH = persist.tile([C, Fh], f32)
                nc.vector.tensor_scalar(avgH[:], avg[:, 0:Fh], rHc[0:C, :], None, OP.mult)
                nc.vector.scalar_tensor_tensor(avgH[:], avg[:, Fh:F], rH[0:C, :], avgH[:], OP.mult, OP.add)
                avgHb = persist.tile([C, Fh], bf16)
                nc.vector.tensor_copy(avgHb[:], avgH[:])
                ps_avt = psum.tile([Fh, C], f32, tag="mm")
                nc.tensor.transpose(ps_avt[:], avgH[:], ident[0:C, 0:C])
                avgHT = persist.tile([Fh, C], f32)
                nc.vector.tensor_copy(avgHT[:], ps_avt[:])

                # Ei[i, f] = exp(he_i[f] * avg[lab_i, f] / tau)  as [K, Fh]
                ps_a = psum.tile([Fh, K], f32, tag="mm2")
                nc.tensor.matmul(ps_a[:], avgHb[:], Mpb[:], start=True, stop=True)
                prodT = persist.tile([Fh, K], f32)
                nc.vector.tensor_tensor(prodT[:], hh[:], ps_a[:], OP.mult)
                ps_pt = psum.tile([K, Fh], f32, tag="mm")
                nc.tensor.transpose(ps_pt[:], prodT[:], ident[0:Fh, 0:Fh])
                Ei = persist.tile([K, Fh], f32)
                nc.scalar.activation(Ei[:], ps_pt[:], AF.Exp, scale=1.0 / TAU)

                # pair term, two phases to avoid ACT table thrash:
                #   phase A (all Exp): Ep[c, f*K+j] = exp(avg[c,f] he[f,j] / tau)
                #   phase B (all Ln):  neg = -(Mm1^T @ Ep);  term = ln(Ei + neg)
                Ep = persist.tile([C, Fh * K], bf16)
                for q in range(NQ):
                    sl = slice(q * FQ * K, (q + 1) * FQ * K)
                    nc.vector.tensor_tensor(
                        rap(Ep[:, sl], [[K, FQ], [1, K]]),
                        rap(he_rep[:, sl], [[K, FQ], [1, K]]),
                        rap(avgH[:, q * FQ:(q + 1) * FQ], [[1, FQ], [0, K]]), OP.mult)
                    nc.scalar.activation(Ep[:, sl], Ep[:, sl], AF.Exp, scale=1.0 / TAU)
                S2cols = persist.tile([K, NQ], f32)
                for q in range(NQ):
                    ps_pr = psx.tile([K, FQ * K], f32, tag="pr")
                    # psum-bank limit: a single matmul's out must stay in one
                    # 512-f32 bank, so split the 800-col chunk at 512
                    for lo, hi in ((0, 512), (512, FQ * K)):
                        nc.tensor.matmul(ps_pr[:, lo:hi], Mm1[:],
                                         Ep[:, q * FQ * K + lo:q * FQ * K + hi],
                                         start=True, stop=True)
                    termin = small.tile([K, FQ * K], f32, tag="termin")
                    nc.vector.tensor_tensor(rap(termin[:], [[K, FQ], [1, K]]),
                                            rap(Ei[:, q * FQ:(q + 1) * FQ], [[1, FQ], [0, K]]),
                                            rap(ps_pr[:], [[K, FQ], [1, K]]), OP.subtract)
                    termf = small.tile([K, FQ * K], f32, tag="termf")
                    nc.scalar.activation(termf[:], termin[:], AF.Ln)
                    scr = small.tile([K, FQ * K], f32, tag="scr")
                    nc.vector.tensor_tensor(rap(scr[:], [[K, FQ], [1, K]]),
                                            rap(mask2[:], [[0, FQ], [1, K]]),
                                            rap(termf[:], [[K, FQ], [1, K]]), OP.mult)
                    nc.vector.tensor_reduce(S2cols[:, q:q + 1], scr[:], AX.X, OP.add)
                S2 = small.tile([K, 1], f32, tag="S2")
                nc.vector.tensor_reduce(S2[:], S2cols[:], AX.X, OP.add)

                ps_u = psum.tile([K, 2], f32, tag="mm")
                nc.tensor.matmul(ps_u[:], Mp[:], rhs2[:], start=True, stop=True)
                U = small.tile([K, 2], f32, tag="U")
                nc.vector.tensor_copy(U[:], ps_u[:])
                ps_g = psum.tile([K, C], f32, tag="mm2")
                nc.tensor.matmul(ps_g[:], hh[:], avgHT[:], start=True, stop=True)
                Gm = small.tile([K, C], f32, tag="Gm")
                nc.vector.tensor_tensor(Gm[:], ps_g[:], M_kc[:], OP.mult)
                li = small.tile([K, 1], f32, tag="li")
                nc.vector.tensor_reduce(li[:], Gm[:], AX.X, OP.add)

                t1 = small.tile([K, 1], f32, tag="t1")
                nc.vector.tensor_tensor(t1[:], S2[:], U[:, 0:1], OP.mult)
                t2 = small.tile([K, 1], f32, tag="t2")
                nc.vector.tensor_tensor(t2[:], li[:], U[:, 1:2], OP.mult)
                cvec = small.tile([K, 1], f32, tag="cvec")
                nc.vector.scalar_tensor_tensor(cvec[:], t2[:], -1.0 / TAU, t1[:], OP.mult, OP.add)

                ps_t = psum.tile([1, 1], f32, tag="mm")
                nc.tensor.matmul(ps_t[:], cvec[:], ones128[0:K, :], start=True, stop=True)
                lossp = small.tile([1, 8], f32, tag="lossp")
                nc.vector.memset(lossp[:], 0.0)
                nc.vector.tensor_scalar(lossp[:, 0:1], ps_t[:], -1.0 / B, None, OP.mult)
                nc.sync.dma_start(lin[:], lossp[:])
                if NOCOLL:
                    nc.gpsimd.dma_start(AP(lout, 0, [[8, 8], [1, 8]]),
                                        AP(lin, 0, [[0, 8], [1, 8]]))
                else:
                    nc.gpsimd.collective_compute("AllGather", OP.bypass, ins=[lin[:]], outs=[lout[:]], replica_groups=ALL)
                res8 = small.tile([1, 8], f32, tag="res8")
                nc.sync.dma_start(res8[:], AP(lout, 0, [[0, 1], [8, 8]]))
                res = small.tile([1, 1], f32, tag="res")
                nc.vector.tensor_reduce(res[:], res8[:], AX.X, OP.add)
                nc.sync.dma_start(out[:], res[:])
            else:
                resd = small.tile([1, 1], f32, tag="resd")
                nc.sync.dma_start(resd[:], AP(sumout, 0, [[1, 1], [1, 1]]))
                nc.sync.dma_start(out[:], resd[:])

    nc.compile()
    ctx.close()
    return nc


def make_in_maps(proba, y, embeddings):
    import ml_dtypes
    bf = ml_dtypes.bfloat16
    in_maps = []
    for core in range(NCORES):
        b, h = core // 2, core % 2
        sl = slice(h * Vh, (h + 1) * Vh)
        pT = np.ascontiguousarray(proba[b, :, sl].T.astype(bf))
        yT = np.ascontiguousarray(y[b, :, sl].T.astype(bf))
        eT = np.ascontiguousarray(
            np.concatenate([embeddings[b, :, sl].T,
                            np.ones((Vh, 1), np.float32)], axis=1).astype(bf))
        ga = np.ascontiguousarray(
            np.concatenate([embeddings[b].T, y[b].T], axis=1).astype(np.float32))
        rl = np.array([[h, 1 - h, h * Vh, (1 - h) * Vh, 0, 0, 0, 0]], dtype=np.float32)
        in_maps.append({"pt": pT, "yt": yT, "et": eT, "garr": ga, "role": rl})
    return in_maps


_NC = None


def kernel(proba, y, embeddings):
    global _NC
    from concourse.bass_utils import run_bass_kernel_spmd

    if _NC is None:
        _NC = build_program()
    in_maps = make_in_maps(proba, y, embeddings)
    res = run_bass_kernel_spmd(_NC, in_maps, core_ids=list(range(NCORES)))
    return np.float32(res.results[0]["out"].reshape(())).reshape(())
